# revision 1
# baseline (speedup 1.0000x reference)
"""Trainium2 Bass kernel for CISTransformerDecoder (deformable-attention decoder).

Sharding: 8 cores = 2 batches x 4 head-groups (2 heads each).
Each core materializes per-(layer,head) bf16 gather tables (4 shifted copies,
256B units), runs all 1024 queries for its 2 heads, exchanges partial
attention outputs via 4-core AllReduce, and replicates LN/FFN.
"""
import sys, math, contextlib
sys.path.insert(0, "/opt/trn_rl_repo")
import numpy as np

import concourse.bass as bass
import concourse.bacc as bacc
import concourse.tile as tile
from concourse import mybir
from concourse import library_config
from concourse.bass_utils import run_bass_kernel_spmd
from concourse.masks import make_identity

F32, BF16, I32, I16 = (mybir.dt.float32, mybir.dt.bfloat16, mybir.dt.int32,
                       mybir.dt.int16)
AF = mybir.ActivationFunctionType
AL = mybir.AluOpType
AX = mybir.AxisListType

SHAPES = ((128, 128), (64, 64), (32, 32), (16, 16))
NL, NH, NP, D, HD = 4, 8, 4, 256, 32
NLAYERS, DFF, BS, NQ = 6, 1024, 2, 1024
LEN = sum(h * w for h, w in SHAPES)        # 21760
NCORES, HPC = 8, 2
NQB = NQ // 128                             # 8
KP = NL * NP * 2                            # 32 gather units per (q, head)
RPC = 5441                                  # table rows per copy
TROWS = 4 * RPC                             # 21764
ELEM = 128                                  # bf16 elems per unit (256B)
SMAX = float(LEN)
LVL_W = [w for (h, w) in SHAPES]
LVL_H = [h for (h, w) in SHAPES]
LVL_BASE = [1, 1 + 16384, 1 + 16384 + 4096, 1 + 16384 + 4096 + 1024]

_CACHE = {}
SKIP_GATHER = False
SKIP_CC = False


def _ap(t, off, dims):
    return bass.AP(t.tensor, t.offset + off, dims)


def _p0(t):
    return list(t.ap[0])



def _qmaj(dram_t, n):
    # DRAM AP for [(a p), n] laid out query-major, enumerated as (p, a, n)
    t = dram_t if isinstance(dram_t, bass.AP) else dram_t.ap()
    return bass.AP(t.tensor, t.offset, [[n, 128], [128 * n, NQB], [1, n]])

def build_nc(debug=False):
    nc = bacc.Bacc("TRN2", target_bir_lowering=False, debug=False,
                   num_devices=NCORES)
    dt = nc.dram_tensor
    ins = {}
    ins["tgts"] = dt("tgts", [NQ, D], F32, kind="ExternalInput")
    ins["refc"] = dt("refc", [4, NQ], F32, kind="ExternalInput")
    for l in range(NL):
        hw = LVL_H[l] * LVL_W[l]
        ins[f"mem{l}"] = dt(f"mem{l}", [D, hw], F32, kind="ExternalInput")
        ins[f"pm{l}"] = dt(f"pm{l}", [D, hw], F32, kind="ExternalInput")
    ins["lemb"] = dt("lemb", [NL, D], F32, kind="ExternalInput")
    for nm in ("ax", "bx", "ay", "by"):
        ins[nm] = dt(nm, [NQ, 32], F32, kind="ExternalInput")
    ins["cw"] = dt("cw", [6, 32], F32, kind="ExternalInput")
    ins["ds"] = dt("ds", [128, 2], F32, kind="ExternalInput")
    ins["rp_w1"] = dt("rp_w1", [512, D], F32, kind="ExternalInput")
    ins["rp_b1"] = dt("rp_b1", [D], F32, kind="ExternalInput")
    ins["rp_w2"] = dt("rp_w2", [D, D], F32, kind="ExternalInput")
    ins["rp_b2"] = dt("rp_b2", [D], F32, kind="ExternalInput")
    ins["soaw_w"] = dt("soaw_w", [NLAYERS, D, 96], F32, kind="ExternalInput")
    ins["soaw_b"] = dt("soaw_b", [NLAYERS, 96], F32, kind="ExternalInput")
    ins["vp_w"] = dt("vp_w", [NLAYERS, D, HPC * HD], BF16, kind="ExternalInput")
    ins["opw_aug"] = dt("opw_aug", [NLAYERS, 68, D], F32, kind="ExternalInput")
    ins["f1_w"] = dt("f1_w", [NLAYERS, D, DFF], BF16, kind="ExternalInput")
    ins["f1_b"] = dt("f1_b", [NLAYERS, DFF], F32, kind="ExternalInput")
    ins["f2_w"] = dt("f2_w", [NLAYERS, DFF, D], BF16, kind="ExternalInput")
    ins["f2_b"] = dt("f2_b", [NLAYERS, D], F32, kind="ExternalInput")
    for nm in ("n1_s", "n1_b", "n2_s", "n2_b"):
        ins[nm] = dt(nm, [NLAYERS, D], F32, kind="ExternalInput")

    out = dt("out", [NQ, D], F32, kind="ExternalOutput")
    dbg = {}
    if debug:
        for nm, shp in [("d_so", [NQ, 64]), ("d_aw", [NQ, 32]),
                        ("d_x", [NQ, 32]), ("d_y", [NQ, 32]),
                        ("d_s0", [NQ, 32]), ("d_w", [NQ, 128]),
                        ("d_samp", [NQ, 68]), ("d_attn", [NQ, D]),
                        ("d_src1", [NQ, D]), ("d_qpos", [NQ, D]),
                        ("d_val", [128, ELEM]), ("d_idx", [NQ, 64])]:
            dbg[nm] = dt(nm, shp, F32, kind="ExternalOutput")

    groups = [[0, 1, 2, 3], [4, 5, 6, 7]]

    with tile.TileContext(nc) as tc:
        _build_body(nc, tc, ins, out, dbg, groups, debug)
    nc.compile()
    return nc


def _build_body(nc, tc, ins, out, dbg, groups, debug):
    ctx = contextlib.ExitStack()
    consts = ctx.enter_context(tc.tile_pool(name="consts", bufs=1))
    persist = ctx.enter_context(tc.tile_pool(name="persist", bufs=1))
    dpool = ctx.enter_context(tc.tile_pool(name="dpool", bufs=1, space="DRAM"))
    dwork = ctx.enter_context(tc.tile_pool(name="dwork", bufs=2, space="DRAM"))
    tables = [[dpool.tile([TROWS, ELEM], BF16, tag=f"tbl_{i}_{h}",
                          name=f"tbl_{i}_{h}") for h in range(HPC)]
              for i in range(NLAYERS)]

    nc.gpsimd.load_library(library_config.mlp)

    ident = consts.tile([128, 128], F32)
    make_identity(nc, ident[:])

    # const tiles
    cw = consts.tile([128, 6, 32], F32)
    nc.sync.dma_start(out=cw[:], in_=bass.AP(ins["cw"].ap().tensor, 0,
                                             [[0, 128], [32, 6], [1, 32]]))
    locc = consts.tile([128, 4, NQB, 32], F32)
    for ci, nm in enumerate(("ax", "bx", "ay", "by")):
        nc.sync.dma_start(out=locc[:, ci], in_=bass.AP(
            ins[nm].ap().tensor, 0, [[32, 128], [128 * 32, NQB], [1, 32]]))
    lemb_t = consts.tile([128, NL, 2], F32)
    nc.sync.dma_start(out=lemb_t[:], in_=bass.AP(
        ins["lemb"].ap().tensor, 0, [[1, 128], [D, NL], [128, 2]]))

    # ================= phase 0: query_pos (transposed) =================
    qposT = [persist.tile([128, NQ], F32, tag=f"qposT{m}", name=f"qposT{m}") for m in range(2)]
    with tc.tile_pool(name="ph0", bufs=1) as ph0, \
         tc.tile_pool(name="ph0p", bufs=2, space="PSUM") as ph0p:
        ds = ph0.tile([128, 2], F32)
        nc.sync.dma_start(out=ds[:], in_=ins["ds"][:, :])
        qsT = [ph0.tile([128, NQ], F32, tag=f"qsT{c}", name=f"qsT{c}") for c in range(4)]
        for c in range(4):
            vrow = ph0.tile([128, NQ], F32, tag="vrow")
            nc.sync.dma_start(out=vrow[:], in_=bass.AP(
                ins["refc"].ap().tensor, c * NQ, [[0, 128], [1, NQ]]))
            sg = ph0.tile([128, NQ], F32, tag="sg")
            nc.scalar.activation(out=sg[:], in_=vrow[:], func=AF.Sigmoid)
            nc.scalar.activation(out=qsT[c][:], in_=sg[:], func=AF.Sin,
                                 scale=ds[:, 0:1], bias=ds[:, 1:2])
        rpb1 = ph0.tile([128, 2], F32)
        nc.sync.dma_start(out=rpb1[:], in_=bass.AP(
            ins["rp_b1"].ap().tensor, 0, [[1, 128], [128, 2]]))
        rpb2 = ph0.tile([128, 2], F32)
        nc.sync.dma_start(out=rpb2[:], in_=bass.AP(
            ins["rp_b2"].ap().tensor, 0, [[1, 128], [128, 2]]))
        w1t = ph0.tile([128, 4, D], F32)
        nc.sync.dma_start(out=w1t[:], in_=ins["rp_w1"][:, :].rearrange(
            "(c p) d -> p c d", p=128))
        w2t = ph0.tile([128, 2, D], F32)
        nc.sync.dma_start(out=w2t[:], in_=ins["rp_w2"][:, :].rearrange(
            "(c p) d -> p c d", p=128))
        h1T = [ph0.tile([128, NQ], F32, tag=f"h1T{m}", name=f"h1T{m}") for m in range(2)]
        for m in range(2):
            for qc in range(2):
                pt = ph0p.tile([128, 512], F32, tag="pmlp", name="pmlp")
                for c in range(4):
                    nc.tensor.matmul(out=pt[:],
                                     lhsT=w1t[:, c, m * 128:(m + 1) * 128],
                                     rhs=qsT[c][:, qc * 512:(qc + 1) * 512],
                                     start=(c == 0), stop=(c == 3))
                nc.scalar.activation(out=h1T[m][:, qc * 512:(qc + 1) * 512],
                                     in_=pt[:], func=AF.Relu,
                                     bias=rpb1[:, m:m + 1])
        for m in range(2):
            for qc in range(2):
                pt = ph0p.tile([128, 512], F32, tag="pmlp", name="pmlp")
                for c in range(2):
                    nc.tensor.matmul(out=pt[:],
                                     lhsT=w2t[:, c, m * 128:(m + 1) * 128],
                                     rhs=h1T[c][:, qc * 512:(qc + 1) * 512],
                                     start=(c == 0), stop=(c == 1))
                # qposT = psum + rp_b2 (per-partition)
                nc.vector.tensor_scalar(
                    out=qposT[m][:, qc * 512:(qc + 1) * 512], in0=pt[:],
                    scalar1=rpb2[:, m:m + 1], scalar2=None, op0=AL.add)
    if debug:
        with tc.tile_pool(name="dbg0", bufs=1) as dbg0, \
             tc.tile_pool(name="dbg0p", bufs=2, space="PSUM") as dbg0p:
            qpd = dbg0.tile([128, NQB, D], F32, name="qpd")
            for m in range(2):
                for a in range(NQB):
                    pt = dbg0p.tile([128, 128], F32, tag="ptr", name="ptr")
                    nc.tensor.transpose(out=pt[:],
                                        in_=qposT[m][:, a * 128:(a + 1) * 128],
                                        identity=ident[:])
                    nc.scalar.activation(out=qpd[:, a, m * 128:(m + 1) * 128],
                                         in_=pt[:], func=AF.Copy)
            nc.sync.dma_start(out=_qmaj(dbg["d_qpos"], D), in_=qpd[:])

    # ================= phase 1: value_in + gather tables =================
    vpw = consts.tile([128, NLAYERS, 2, HPC * HD], BF16)
    nc.sync.dma_start(out=vpw[:], in_=ins["vp_w"][:, :, :].rearrange(
        "l (c p) n -> p l c n", p=128))
    ztile = consts.tile([128, HD], BF16)
    nc.vector.memset(ztile[:], 0.0)
    with tc.tile_pool(name="vin", bufs=1) as vin_pool, \
         tc.tile_pool(name="vwork", bufs=3) as vwork, \
         tc.tile_pool(name="vpsum", bufs=4, space="PSUM") as vpsum:
        vin = [vin_pool.tile([128, LEN], BF16, tag=f"vin{c}", name=f"vin{c}") for c in range(2)]
        for l in range(NL):
            hw = LVL_H[l] * LVL_W[l]
            base = LVL_BASE[l] - 1
            for c in range(2):
                for o in range(0, hw, 2048):
                    wch = min(2048, hw - o)
                    mt = vwork.tile([128, 2048], F32, tag="mt", name="mt")
                    nc.sync.dma_start(out=mt[:, :wch],
                                      in_=ins[f"mem{l}"][c * 128:(c + 1) * 128,
                                                         o:o + wch])
                    nc.gpsimd.dma_start(out=mt[:, :wch],
                                        in_=ins[f"pm{l}"][c * 128:(c + 1) * 128,
                                                          o:o + wch],
                                        accum_op=AL.add)
                    nc.scalar.activation(out=vin[c][:, base + o:base + o + wch],
                                         in_=mt[:, :wch], func=AF.Identity,
                                         bias=lemb_t[:, l, c:c + 1])
        for i in range(NLAYERS):
            for s0 in range(0, LEN, 1024):
                nb = min(1024, LEN - s0) // 128
                pt = vpsum.tile([128, 8, HPC * HD], F32, tag="vp", name="vp")
                for t in range(nb):
                    for c in range(2):
                        lhsT = _ap(vin[c], s0 + t, [_p0(vin[c]), [nb, 128]])
                        nc.tensor.matmul(out=pt[:, t], lhsT=lhsT,
                                         rhs=vpw[:, i, c, :],
                                         start=(c == 0), stop=(c == 1))
                st = vwork.tile([128, 8, HPC * HD], BF16, tag="st", name="st")
                nc.scalar.activation(out=st[:, :nb], in_=pt[:, :nb],
                                     func=AF.Identity)
                for h in range(HPC):
                    for cp in range(4):
                        off = (s0 + 1 - cp) * HD + cp * RPC * ELEM
                        nc.sync.dma_start(
                            out=bass.AP(tables[i][h].tensor, off,
                                        [[nb * HD, 128], [HD, nb], [1, HD]]),
                            in_=st[:, :nb, h * HD:(h + 1) * HD])
            for h in range(HPC):
                for cp in range(4):
                    if cp == 0:
                        nc.sync.dma_start(
                            out=bass.AP(tables[i][h].tensor, 0,
                                        [[HD, 1], [1, HD]]),
                            in_=ztile[:1, :])
                    lo = LEN + 1
                    n_pad = cp + 3
                    nc.sync.dma_start(
                        out=bass.AP(tables[i][h].tensor,
                                    cp * RPC * ELEM + (lo - cp) * HD,
                                    [[HD, n_pad], [1, HD]]),
                        in_=ztile[:n_pad, :])

    # ================= main-loop pools + weights =================
    lwork = ctx.enter_context(tc.tile_pool(name="lwork", bufs=1))
    big = ctx.enter_context(tc.tile_pool(name="big", bufs=2))
    psum = ctx.enter_context(tc.tile_pool(name="psum", bufs=2, space="PSUM"))
    soaw_w = consts.tile([128, NLAYERS, 2, 96], F32)
    nc.sync.dma_start(out=soaw_w[:], in_=ins["soaw_w"][:, :, :].rearrange(
        "l (c p) n -> p l c n", p=128))
    soaw_b = consts.tile([128, NLAYERS, 96], F32)
    nc.sync.dma_start(out=soaw_b[:], in_=bass.AP(
        ins["soaw_b"].ap().tensor, 0, [[0, 128], [96, NLAYERS], [1, 96]]))
    opw = consts.tile([68, NLAYERS, D], F32)
    nc.sync.dma_start(out=opw[:], in_=ins["opw_aug"][:, :, :].rearrange(
        "l p n -> p l n"))
    f1bT = consts.tile([128, NLAYERS, 8], F32)
    nc.sync.dma_start(out=f1bT[:], in_=bass.AP(
        ins["f1_b"].ap().tensor, 0, [[1, 128], [DFF, NLAYERS], [128, 8]]))
    fwins = (ins["f1_w"], ins["f2_w"])
    fbins = {nm: ins[nm] for nm in ("f2_b", "n1_s", "n1_b", "n2_s", "n2_b")}

    src = persist.tile([128, NQB, D], F32)
    nc.sync.dma_start(out=src[:], in_=ins["tgts"][:, :].rearrange(
        "(a p) d -> p a d", p=128))
    eps = consts.tile([128, 1], F32)
    nc.vector.memset(eps[:], 1e-5)

    for li in range(NLAYERS):
        _layer(nc, li, src, qposT, soaw_w, soaw_b, opw, fwins, f1bT, fbins,
               cw, locc, ident, eps, tables[li], dwork,
               groups, lwork, big, psum, dbg, debug and li == 0)

    nc.sync.dma_start(out=_qmaj(out, D), in_=src[:])
    if debug:
        nc.gpsimd.dma_start(out=dbg["d_val"][:, :],
                            in_=bass.AP(tables[0][0].tensor, 0,
                                        [[ELEM, 128], [1, ELEM]]))
    ctx.close()


def _layer(nc, li, src, qposT, soaw_w, soaw_b, opw, fwins, f1bT, fbins,
           cw, locc, ident, eps, tbls, dwork, groups,
           lwork, big, psum, dbg, dump):
    P4 = [128, NQB, 2, 16]
    # per-layer FFN weights + bias rows
    f1w = lwork.tile([128, 2, DFF], BF16, tag="f1w", name="f1w")
    nc.sync.dma_start(out=f1w[:], in_=fwins[0][li, :, :].rearrange(
        "(c p) n -> p c n", p=128))
    f2w = lwork.tile([128, 8, D], BF16, tag="f2w", name="f2w")
    nc.sync.dma_start(out=f2w[:], in_=fwins[1][li, :, :].rearrange(
        "(c p) n -> p c n", p=128))
    fb = {}
    for nm, dr in fbins.items():
        fb[nm] = lwork.tile([128, D], F32, tag=f"fb_{nm}", name=f"fb_{nm}")
        nc.sync.dma_start(out=fb[nm][:], in_=bass.AP(
            dr.ap().tensor, li * D, [[0, 128], [1, D]]))

    def T(tag, shape=None, dtp=F32):
        return lwork.tile(shape or P4, dtp, tag=tag, name=tag)

    def tt(o, i0, i1, op):
        nc.any.tensor_tensor(out=o, in0=i0, in1=i1, op=op)

    # ---- qT = srcT + qposT ----
    qT = [T(f"qT{m}", [128, NQ]) for m in range(2)]
    for a in range(NQB):
        for m in range(2):
            pt = psum.tile([128, 128], F32, tag="ptr", name="ptr")
            nc.tensor.transpose(out=pt[:], in_=src[:, a, m * 128:(m + 1) * 128],
                                identity=ident[:])
            nc.scalar.activation(out=qT[m][:, a * 128:(a + 1) * 128],
                                 in_=pt[:], func=AF.Copy)
    for m in range(2):
        tt(qT[m][:], qT[m][:], qposT[m][:], AL.add)

    # ---- so/aw ----
    soaw = T("soaw", [128, NQB, 96])
    for a in range(NQB):
        pt = psum.tile([128, 96], F32, tag="p256", name="p256")
        for m in range(2):
            nc.tensor.matmul(out=pt[:], lhsT=qT[m][:, a * 128:(a + 1) * 128],
                             rhs=soaw_w[:, li, m, :], start=(m == 0),
                             stop=(m == 1))
        nc.vector.tensor_tensor(out=soaw[:, a], in0=pt[:], in1=soaw_b[:, li],
                                op=AL.add)
    if dump:
        nc.sync.dma_start(out=_qmaj(dbg["d_so"], 64), in_=_ap(
            soaw, 0, [_p0(soaw), [96, NQB], [1, 64]]))

    # ---- softmax over 16 per (q, h) ----
    aw = T("aw")
    mx = T("mx", [128, NQB, 2])
    awl = _ap(soaw, 64, [_p0(soaw), [96, NQB], [16, 2], [1, 16]])
    nc.vector.tensor_reduce(out=mx[:], in_=awl, axis=AX.X, op=AL.max)
    tt(aw[:], awl, _ap(mx, 0, [_p0(mx), [2, NQB], [1, 2], [0, 16]]),
       AL.subtract)
    nc.scalar.activation(out=aw[:], in_=aw[:], func=AF.Exp)
    sm = T("sm", [128, NQB, 2])
    nc.vector.tensor_reduce(out=sm[:], in_=aw[:], axis=AX.X, op=AL.add)
    nc.vector.reciprocal(out=sm[:], in_=sm[:])
    tt(aw[:], aw[:], _ap(sm, 0, [_p0(sm), [2, NQB], [1, 2], [0, 16]]), AL.mult)
    if dump:
        nc.sync.dma_start(out=_qmaj(dbg["d_aw"], 32), in_=_ap(
            aw, 0, [_p0(aw), [32, NQB], [1, 32]]))

    # ---- pixel coords ----
    sox = _ap(soaw, 0, [_p0(soaw), [96, NQB], [32, 2], [2, 16]])
    soy = _ap(soaw, 1, [_p0(soaw), [96, NQB], [32, 2], [2, 16]])

    def lc(ci):
        return _ap(locc, ci * NQB * 32,
                   [_p0(locc), [32, NQB], [0, 2], [1, 16]])

    def cwv(r):
        return _ap(cw, r * 32, [_p0(cw), [0, NQB], [16, 2], [1, 16]])

    x, y = T("x"), T("y")
    tt(x[:], sox, lc(1), AL.mult)
    tt(x[:], x[:], lc(0), AL.add)
    tt(y[:], soy, lc(3), AL.mult)
    tt(y[:], y[:], lc(2), AL.add)
    if dump:
        nc.sync.dma_start(out=_qmaj(dbg["d_x"], 32), in_=_ap(
            x, 0, [_p0(x), [32, NQB], [1, 32]]))
        nc.sync.dma_start(out=_qmaj(dbg["d_y"], 32), in_=_ap(
            y, 0, [_p0(y), [32, NQB], [1, 32]]))

    ti = T("ti", P4, I32)
    tf = T("tf")
    fx, fy = T("fx"), T("fy")

    def floor_(dst, xin):
        nc.vector.tensor_copy(out=ti[:], in_=xin)
        nc.vector.tensor_copy(out=tf[:], in_=ti[:])
        nc.vector.tensor_tensor(out=dst[:], in0=tf[:], in1=xin, op=AL.is_gt)
        nc.vector.tensor_tensor(out=dst[:], in0=tf[:], in1=dst[:],
                                op=AL.subtract)

    floor_(fx, x[:])
    floor_(fy, y[:])
    lx, ly = T("lx"), T("ly")
    tt(lx[:], x[:], fx[:], AL.subtract)
    tt(ly[:], y[:], fy[:], AL.subtract)

    t1 = T("t1")
    ix0, ix1, iy0, iy1 = T("iy0"), T("iy1"), T("iy2"), T("iy3")
    for (dst, f, hi) in ((ix0, fx, 3), (iy0, fy, 4)):
        nc.any.tensor_scalar(out=dst[:], in0=f[:], scalar1=0.0, scalar2=None,
                             op0=AL.is_ge)
        tt(t1[:], f[:], cwv(hi), AL.is_le)
        tt(dst[:], dst[:], t1[:], AL.mult)
    for (dst, f, hi) in ((ix1, fx, 5), (iy1, fy, -1)):
        nc.any.tensor_scalar(out=dst[:], in0=f[:], scalar1=-1.0, scalar2=None,
                             op0=AL.is_ge)
        if hi >= 0:
            tt(t1[:], f[:], cwv(hi), AL.is_le)          # fx <= W-2
        else:
            nc.any.tensor_scalar(out=t1[:], in0=f[:], scalar1=1.0,
                                 scalar2=None, op0=AL.add)
            tt(t1[:], t1[:], cwv(4), AL.is_le)          # fy+1 <= H-1
        tt(dst[:], dst[:], t1[:], AL.mult)

    wy0, wy1, wx0, wx1 = T("wy0"), T("wy1"), T("ix0"), T("ix1")
    nc.any.tensor_scalar(out=wy0[:], in0=ly[:], scalar1=-1.0, scalar2=-1.0,
                         op0=AL.add, op1=AL.mult)       # (ly-1)*-1
    tt(wy0[:], wy0[:], iy0[:], AL.mult)
    tt(wy0[:], wy0[:], aw[:], AL.mult)
    tt(wy1[:], ly[:], iy1[:], AL.mult)
    tt(wy1[:], wy1[:], aw[:], AL.mult)
    nc.any.tensor_scalar(out=wx0[:], in0=lx[:], scalar1=-1.0, scalar2=-1.0,
                         op0=AL.add, op1=AL.mult)
    tt(wx0[:], wx0[:], ix0[:], AL.mult)
    tt(wx1[:], lx[:], ix1[:], AL.mult)

    wall = T("wall", [128, NQB, 2, 16, 2, 2], BF16)
    for r, wy in ((0, wy0), (1, wy1)):
        for c, wx in ((0, wx0), (1, wx1)):
            tt(t1[:], wy[:], wx[:], AL.mult)
            nc.vector.tensor_copy(out=_ap(
                wall, r * 2 + c,
                [_p0(wall), [128, NQB], [64, 2], [4, 16]]), in_=t1[:])
    if dump:
        nc.gpsimd.dma_start(out=_qmaj(dbg["d_w"], 128), in_=_ap(
            wall, 0, [_p0(wall), [128, NQB], [1, 128]]))

    samp = T("samp", [128, NQB, 68])
    nc.vector.tensor_reduce(out=_ap(samp, 64, [_p0(samp), [68, NQB], [1, 2]]),
                            in_=_ap(wall, 0, [_p0(wall), [128, NQB], [64, 2],
                                              [1, 64]]),
                            axis=AX.X, op=AL.add)
    nc.vector.memset(samp[:, :, 66:68], 1.0)

    # ---- flat index -> table rows ----
    s0, s1 = T("x"), T("y")  # reuse dead slots
    tt(s0[:], fy[:], cwv(0), AL.mult)
    tt(s0[:], s0[:], fx[:], AL.add)
    tt(s0[:], s0[:], cwv(2), AL.add)
    tt(s1[:], s0[:], cwv(0), AL.add)
    if dump:
        nc.sync.dma_start(out=_qmaj(dbg["d_s0"], 32), in_=_ap(
            s0, 0, [_p0(s0), [32, NQB], [1, 32]]))
    idxe = T("idxe", [128, NQB, 2, 16, 2])
    for r, s in ((0, s0), (1, s1)):
        nc.any.tensor_scalar(out=s[:], in0=s[:], scalar1=0.0, scalar2=SMAX,
                             op0=AL.max, op1=AL.min)
        nc.any.tensor_scalar(out=t1[:], in0=s[:], scalar1=0.25, scalar2=-0.375,
                             op0=AL.mult, op1=AL.add)
        nc.vector.tensor_copy(out=ti[:], in_=t1[:])
        nc.vector.tensor_copy(out=t1[:], in_=ti[:])
        # c = s - 4*floor4 in [0,3]; idx = c*RPC + floor4  (all exact in f32)
        nc.vector.scalar_tensor_tensor(out=tf[:], in0=t1[:], scalar=-4.0,
                                       in1=s[:], op0=AL.mult, op1=AL.add)
        nc.vector.scalar_tensor_tensor(
            out=_ap(idxe, r, [_p0(idxe), [64, NQB], [32, 2], [2, 16]]),
            in0=tf[:], scalar=float(RPC), in1=t1[:],
            op0=AL.mult, op1=AL.add)
    if dump:
        nc.sync.dma_start(out=_qmaj(dbg["d_idx"], 64), in_=_ap(
            idxe, 0, [_p0(idxe), [64, NQB], [1, 64]]))

    # ---- idx wrap bridge ----
    idxT = T("qT0", [128, NQ])
    for a in range(NQB):
        pt = psum.tile([128, 128], F32, tag="ptr", name="ptr")
        nc.tensor.transpose(out=pt[:64, :],
                            in_=_ap(idxe, a * 64, [_p0(idxe), [1, 64]]),
                            identity=ident[:])
        nc.scalar.activation(out=idxT[:64, a * 128:(a + 1) * 128],
                             in_=pt[:64, :], func=AF.Copy)
    st16 = T("st16", [16, NQB, 2, 32, 8], I16)
    for aq in range(NQB):
        for a8 in range(8):
            pt = psum.tile([128, 128], F32, tag="ptr", name="ptr")
            nc.tensor.transpose(
                out=pt[:16, :64],
                in_=idxT[:64, aq * 128 + a8 * 16:aq * 128 + a8 * 16 + 16],
                identity=ident[:64, :64])
            nc.vector.tensor_copy(out=_ap(
                st16, aq * 512 + a8, [_p0(st16), [256, 2], [8, 32]]),
                in_=pt[:16, :64])
    idx_dram = dwork.tile([16, NQB * HPC * KP * 8], I16, tag="idxd",
                          name="idxd")
    nc.sync.dma_start(out=idx_dram[:, :], in_=st16[:].rearrange(
        "p a h k e -> p (a h k e)"))

    # ---- gather + weighted sum ----
    for a in range(NQB):
        for h in range(HPC):
            idx16 = big.tile([128, 256], I16, tag="idx16", name="idx16")
            nc.sync.dma_start(out=idx16[:], in_=_ap(
                idx_dram, a * 512 + h * 256,
                [[0, 8], [4096, 16], [1, 256]]))
            g = big.tile([128, KP, ELEM], BF16, tag="g", name="g")
            if SKIP_GATHER:
                nc.vector.memset(g[:], 0.25)
            else:
                nc.gpsimd.dma_gather(
                    out_ap=g[:], in_ap=tbls[h][:, :], idxs_ap=idx16[:],
                    num_idxs=128 * KP, num_idxs_reg=128 * KP, elem_size=ELEM,
                    single_packet=False)
            m = big.tile([128, KP, 64], BF16, tag="m", name="m")
            nc.any.tensor_tensor(
                out=m[:],
                in0=_ap(g, 0, [_p0(g), [ELEM, KP], [32, 2], [1, 32]]),
                in1=_ap(wall, a * 128 + h * 64,
                        [_p0(wall), [2, KP], [1, 2], [0, 32]]),
                op=AL.mult)
            s = big.tile([128, KP, 32], BF16, tag="s", name="s")
            nc.any.tensor_tensor(out=s[:], in0=m[:, :, 0:32],
                                 in1=m[:, :, 32:64], op=AL.add)
            nc.vector.tensor_reduce(
                out=samp[:, a, h * 32:(h + 1) * 32],
                in_=_ap(s, 0, [_p0(s), [1, 32], [32, KP]]),
                axis=AX.X, op=AL.add)
    if dump:
        nc.sync.dma_start(out=_qmaj(dbg["d_samp"], 68), in_=samp[:])

    # ---- sampledT + op projection + AllReduce ----
    sampT = T("qT1", [68, NQ])
    for a in range(NQB):
        pt = psum.tile([128, 128], F32, tag="ptr", name="ptr")
        nc.tensor.transpose(out=pt[:68, :], in_=samp[:, a, :],
                            identity=ident[:])
        nc.scalar.activation(out=sampT[:, a * 128:(a + 1) * 128],
                             in_=pt[:68, :], func=AF.Copy)
    attn = T("attn", [128, NQB, D])
    for a in range(NQB):
        pt = psum.tile([128, D], F32, tag="p256", name="p256")
        nc.tensor.matmul(out=pt[:], lhsT=sampT[:, a * 128:(a + 1) * 128],
                         rhs=opw[:, li, :], start=True, stop=True)
        nc.scalar.activation(out=attn[:, a], in_=pt[:], func=AF.Copy)
    if not SKIP_CC:
        cc_in = dwork.tile([NQ, D], F32, tag="cc_in", name="cc_in")
        cc_out = dwork.tile([NQ, D], F32, tag="cc_out", name="cc_out")
        nc.sync.dma_start(out=_qmaj(cc_in, D), in_=attn[:])
        nc.gpsimd.collective_compute(
            "AllReduce", AL.add, replica_groups=groups,
            ins=[cc_in[:].opt()], outs=[cc_out[:].opt()])
        nc.sync.dma_start(out=attn[:], in_=_qmaj(cc_out, D))
    if dump:
        nc.sync.dma_start(out=_qmaj(dbg["d_attn"], D), in_=attn[:])

    _layernorm(nc, src, attn, fb["n1_s"], fb["n1_b"], eps, lwork, src)
    if dump:
        nc.sync.dma_start(out=_qmaj(dbg["d_src1"], D), in_=src[:])

    # ---- FFN ----
    s1T = [T(f"s1T{m}", [128, NQ], BF16) for m in range(2)]
    for a in range(NQB):
        for m in range(2):
            pt = psum.tile([128, 128], F32, tag="ptr", name="ptr")
            nc.tensor.transpose(out=pt[:], in_=src[:, a, m * 128:(m + 1) * 128],
                                identity=ident[:])
            nc.scalar.activation(out=s1T[m][:, a * 128:(a + 1) * 128],
                                 in_=pt[:], func=AF.Copy)
    hT = [T(f"hT{m8}", [128, NQ], BF16) for m8 in range(8)]
    for m8 in range(8):
        for qc in range(2):
            pt = psum.tile([128, 512], F32, tag="p512", name="p512")
            for m in range(2):
                nc.tensor.matmul(out=pt[:],
                                 lhsT=f1w[:, m, m8 * 128:(m8 + 1) * 128],
                                 rhs=s1T[m][:, qc * 512:(qc + 1) * 512],
                                 start=(m == 0), stop=(m == 1))
            nc.scalar.activation(out=hT[m8][:, qc * 512:(qc + 1) * 512],
                                 in_=pt[:], func=AF.Relu,
                                 bias=f1bT[:, li, m8:m8 + 1])
    ffn = T("attn", [128, NQB, D])
    for a in range(NQB):
        pt = psum.tile([128, D], F32, tag="p256", name="p256")
        for m8 in range(8):
            nc.tensor.matmul(out=pt[:], lhsT=hT[m8][:, a * 128:(a + 1) * 128],
                             rhs=f2w[:, m8, :], start=(m8 == 0),
                             stop=(m8 == 7))
        nc.vector.tensor_tensor(out=ffn[:, a], in0=pt[:], in1=fb["f2_b"][:],
                                op=AL.add)
    _layernorm(nc, src, ffn, fb["n2_s"], fb["n2_b"], eps, lwork, src)


def _layernorm(nc, src, delta, gamma_t, beta_t, eps, lwork, out_tile):
    # delta <- src + delta;  out_tile <- LN(delta)*gamma + beta
    nc.any.tensor_tensor(out=delta[:], in0=src[:], in1=delta[:], op=AL.add)
    stats = lwork.tile([128, NQB, 6], F32, tag="ln_st", name="ln_st")
    mv = lwork.tile([128, NQB, 2], F32, tag="ln_mv", name="ln_mv")
    for a in range(NQB):
        nc.vector.bn_stats(out=stats[:, a], in_=delta[:, a])
        nc.vector.bn_aggr(out=mv[:, a], in_=stats[:, a])
    rstd = lwork.tile([128, NQB], F32, tag="ln_rs", name="ln_rs")
    nc.scalar.activation(out=rstd[:], in_=_ap(mv, 1, [_p0(mv), [2, NQB]]),
                         func=AF.Sqrt, bias=eps[:, 0:1])
    nc.vector.reciprocal(out=rstd[:], in_=rstd[:])
    nmr = lwork.tile([128, NQB], F32, tag="ln_nm", name="ln_nm")
    nc.vector.tensor_tensor(out=nmr[:], in0=_ap(mv, 0, [_p0(mv), [2, NQB]]),
                            in1=rstd[:], op=AL.mult)
    nc.any.tensor_scalar(out=nmr[:], in0=nmr[:], scalar1=-1.0, scalar2=None,
                         op0=AL.mult)
    for a in range(NQB):
        nc.vector.tensor_scalar(out=delta[:, a], in0=delta[:, a],
                                scalar1=rstd[:, a:a + 1],
                                scalar2=nmr[:, a:a + 1],
                                op0=AL.mult, op1=AL.add)
    g = _ap(gamma_t, 0, [_p0(gamma_t), [0, NQB], [1, D]])
    b = _ap(beta_t, 0, [_p0(beta_t), [0, NQB], [1, D]])
    nc.any.tensor_tensor(out=delta[:], in0=delta[:], in1=g, op=AL.mult)
    nc.any.tensor_tensor(out=out_tile[:], in0=delta[:], in1=b, op=AL.add)


# ======================= host side =======================

def _expand32(v_ql):
    return np.ascontiguousarray(
        np.broadcast_to(v_ql[:, None, :, None], (NQ, HPC, NL, NP))
        .reshape(NQ, 32)).astype(np.float32)


def _host_inputs(inputs):
    import ml_dtypes
    tgts = np.asarray(inputs["tgts"], np.float32)
    refp = np.asarray(inputs["reference_points"], np.float32)
    masks = [np.asarray(inputs[f"mask{l}"]) for l in range(NL)]
    vrs = []
    for m in masks:
        H, W = m.shape[1], m.shape[2]
        vh = (~m[:, :, 0]).sum(1).astype(np.float32) / H
        vw = (~m[:, 0, :]).sum(1).astype(np.float32) / W
        vrs.append(np.stack([vw, vh], -1))
    vr = np.stack(vrs, 1)

    i = np.arange(128, dtype=np.float64)
    dim_t = 10000.0 ** (2 * np.floor(i / 2) / 128)
    ds = np.stack([2 * math.pi / dim_t,
                   np.where(np.arange(128) % 2 == 0, 0.0, math.pi / 2)],
                  -1).astype(np.float32)

    cw = np.zeros((6, 32), np.float32)
    for h2 in range(HPC):
        for l in range(NL):
            sl = slice(h2 * 16 + l * NP, h2 * 16 + (l + 1) * NP)
            cw[0, sl] = LVL_W[l]
            cw[1, sl] = LVL_H[l]
            cw[2, sl] = LVL_BASE[l]
            cw[3, sl] = LVL_W[l] - 1
            cw[4, sl] = LVL_H[l] - 1
            cw[5, sl] = LVL_W[l] - 2

    so_w = np.asarray(inputs["so_w"], np.float32)
    so_b = np.asarray(inputs["so_b"], np.float32)
    aw_w = np.asarray(inputs["aw_w"], np.float32)
    aw_b = np.asarray(inputs["aw_b"], np.float32)
    vp_w = np.asarray(inputs["vp_w"], np.float32)
    vp_b = np.asarray(inputs["vp_b"], np.float32)
    op_w = np.asarray(inputs["op_w"], np.float32)
    op_b = np.asarray(inputs["op_b"], np.float32)
    Wv = np.array(LVL_W, np.float32)[None]
    Hv = np.array(LVL_H, np.float32)[None]

    in_maps = []
    for core in range(NCORES):
        b, hg = core // 4, core % 4
        h0 = 2 * hg
        m = {}
        m["tgts"] = tgts[b]
        m["refc"] = np.ascontiguousarray(refp[b].T[[1, 0, 2, 3]])
        for l in range(NL):
            m[f"mem{l}"] = np.ascontiguousarray(
                np.asarray(inputs[f"mem{l}"], np.float32)[b].reshape(D, -1))
            m[f"pm{l}"] = np.ascontiguousarray(
                np.asarray(inputs[f"pm{l}"], np.float32)[b].reshape(D, -1))
        m["lemb"] = np.asarray(inputs["level_embed"], np.float32)
        ri = refp[b][:, None, :] * np.concatenate([vr[b], vr[b]], -1)[None]
        m["ax"] = _expand32(ri[:, :, 0] * Wv - 0.5)
        m["bx"] = _expand32(ri[:, :, 2] * Wv * (0.5 / NP))
        m["ay"] = _expand32(ri[:, :, 1] * Hv - 0.5)
        m["by"] = _expand32(ri[:, :, 3] * Hv * (0.5 / NP))
        m["cw"] = cw
        m["ds"] = ds
        m["rp_w1"] = np.asarray(inputs["rp_w1"], np.float32)
        m["rp_b1"] = np.asarray(inputs["rp_b1"], np.float32)
        m["rp_w2"] = np.asarray(inputs["rp_w2"], np.float32)
        m["rp_b2"] = np.asarray(inputs["rp_b2"], np.float32)
        so_r = so_w.reshape(NLAYERS, D, NH, NL, NP, 2)
        aw_r = aw_w.reshape(NLAYERS, D, NH, NL, NP)
        sob_r = so_b.reshape(NLAYERS, NH, NL, NP, 2)
        awb_r = aw_b.reshape(NLAYERS, NH, NL, NP)
        m["soaw_w"] = np.ascontiguousarray(np.concatenate([
            so_r[:, :, h0:h0 + 2].reshape(NLAYERS, D, 64),
            aw_r[:, :, h0:h0 + 2].reshape(NLAYERS, D, 32)], -1))
        m["soaw_b"] = np.ascontiguousarray(np.concatenate([
            sob_r[:, h0:h0 + 2].reshape(NLAYERS, 64),
            awb_r[:, h0:h0 + 2].reshape(NLAYERS, 32)], -1))
        m["vp_w"] = np.ascontiguousarray(vp_w[:, :, h0 * HD:(h0 + 2) * HD]).astype(ml_dtypes.bfloat16)
        opa = np.zeros((NLAYERS, 68, D), np.float32)
        for lii in range(NLAYERS):
            opa[lii, 0:64] = op_w[lii, h0 * HD:(h0 + 2) * HD]
            for hh in range(HPC):
                sl = slice((h0 + hh) * HD, (h0 + hh + 1) * HD)
                opa[lii, 64 + hh] = vp_b[lii, sl] @ op_w[lii, sl]
            opa[lii, 66] = op_b[lii] / 4.0
        m["opw_aug"] = opa
        m["f1_w"] = np.asarray(inputs["f1_w"], np.float32).astype(
            ml_dtypes.bfloat16)
        m["f2_w"] = np.asarray(inputs["f2_w"], np.float32).astype(
            ml_dtypes.bfloat16)
        m["f1_b"] = np.asarray(inputs["f1_b"], np.float32)
        m["f2_b"] = np.asarray(inputs["f2_b"], np.float32)
        for nm in ("n1_s", "n1_b", "n2_s", "n2_b"):
            m[nm] = np.asarray(inputs[nm], np.float32)
        in_maps.append(m)
    return in_maps


def kernel(**inputs):
    if "nc" not in _CACHE:
        _CACHE["nc"] = build_nc(debug=False)
    nc = _CACHE["nc"]
    in_maps = _host_inputs(inputs)
    res = run_bass_kernel_spmd(nc, in_maps, list(range(NCORES)))
    return np.stack([res.results[0]["out"],
                     res.results[4]["out"]]).astype(np.float32)



# revision 11
# speedup vs baseline: 1.2195x; 1.2195x over previous
"""Trainium2 Bass kernel for CISTransformerDecoder (deformable-attention decoder).

Sharding: 8 cores = 2 batches x 4 head-groups (2 heads each).
Each core materializes per-(layer,head) bf16 gather tables (4 shifted copies,
256B units), runs all 1024 queries for its 2 heads, exchanges partial
attention outputs via 4-core AllReduce, and replicates LN/FFN.
"""
import sys, math, contextlib
sys.path.insert(0, "/opt/trn_rl_repo")
import numpy as np

import concourse.bass as bass
import concourse.bacc as bacc
import concourse.tile as tile
from concourse import mybir
from concourse import library_config
from concourse.bass_utils import run_bass_kernel_spmd
from concourse.masks import make_identity

F32, BF16, I32, I16 = (mybir.dt.float32, mybir.dt.bfloat16, mybir.dt.int32,
                       mybir.dt.int16)
AF = mybir.ActivationFunctionType
AL = mybir.AluOpType
AX = mybir.AxisListType

SHAPES = ((128, 128), (64, 64), (32, 32), (16, 16))
NL, NH, NP, D, HD = 4, 8, 4, 256, 32
NLAYERS, DFF, BS, NQ = 6, 1024, 2, 1024
LEN = sum(h * w for h, w in SHAPES)        # 21760
NCORES, HPC = 8, 2
NQB = NQ // 128                             # 8
KP = NL * NP * 2                            # 32 gather units per (q, head)
RPC = 5441                                  # table rows per copy
TROWS = 4 * RPC                             # 21764
ELEM = 128                                  # bf16 elems per unit (256B)
SMAX = float(LEN)
LVL_W = [w for (h, w) in SHAPES]
LVL_H = [h for (h, w) in SHAPES]
LVL_BASE = [1, 1 + 16384, 1 + 16384 + 4096, 1 + 16384 + 4096 + 1024]

_CACHE = {}
SKIP_GATHER = False
SKIP_CC = False


def _ap(t, off, dims):
    return bass.AP(t.tensor, t.offset + off, dims)


def _p0(t):
    return list(t.ap[0])



def _qmaj(dram_t, n):
    # DRAM AP for [(a p), n] laid out query-major, enumerated as (p, a, n)
    t = dram_t if isinstance(dram_t, bass.AP) else dram_t.ap()
    return bass.AP(t.tensor, t.offset, [[n, 128], [128 * n, NQB], [1, n]])

def build_nc(debug=False):
    nc = bacc.Bacc("TRN2", target_bir_lowering=False, debug=False,
                   num_devices=NCORES, num_swdge_queues=4)
    dt = nc.dram_tensor
    ins = {}
    ins["tgts"] = dt("tgts", [NQ, D], F32, kind="ExternalInput")
    ins["refc"] = dt("refc", [4, NQ], F32, kind="ExternalInput")
    ins["vin"] = dt("vin", [D, LEN], BF16, kind="ExternalInput")
    for nm in ("ax", "bx", "ay", "by"):
        ins[nm] = dt(nm, [NQ, 32], F32, kind="ExternalInput")
    ins["cw"] = dt("cw", [6, 32], F32, kind="ExternalInput")
    ins["ds"] = dt("ds", [128, 2], F32, kind="ExternalInput")
    ins["rp_w1"] = dt("rp_w1", [512, D], F32, kind="ExternalInput")
    ins["rp_b1"] = dt("rp_b1", [D], F32, kind="ExternalInput")
    ins["rp_w2"] = dt("rp_w2", [D, D], F32, kind="ExternalInput")
    ins["rp_b2"] = dt("rp_b2", [D], F32, kind="ExternalInput")
    ins["soaw_w"] = dt("soaw_w", [NLAYERS, D, 96], F32, kind="ExternalInput")
    ins["soaw_b"] = dt("soaw_b", [NLAYERS, 96], F32, kind="ExternalInput")
    ins["vp_w"] = dt("vp_w", [NLAYERS, D, HPC * HD], BF16, kind="ExternalInput")
    ins["opw_aug"] = dt("opw_aug", [NLAYERS, 68, D], F32, kind="ExternalInput")
    ins["f1_w"] = dt("f1_w", [NLAYERS, D, DFF], BF16, kind="ExternalInput")
    ins["f1_b"] = dt("f1_b", [NLAYERS, DFF], F32, kind="ExternalInput")
    ins["f2_w"] = dt("f2_w", [NLAYERS, DFF, D], BF16, kind="ExternalInput")
    ins["f2_b"] = dt("f2_b", [NLAYERS, D], F32, kind="ExternalInput")
    for nm in ("n1_s", "n1_b", "n2_s", "n2_b"):
        ins[nm] = dt(nm, [NLAYERS, D], F32, kind="ExternalInput")

    out = dt("out", [NQ, D], F32, kind="ExternalOutput")
    dbg = {}
    if debug:
        for nm, shp in [("d_so", [NQ, 64]), ("d_aw", [NQ, 32]),
                        ("d_x", [NQ, 32]), ("d_y", [NQ, 32]),
                        ("d_s0", [NQ, 32]), ("d_w", [NQ, 128]),
                        ("d_samp", [NQ, 68]), ("d_attn", [NQ, D]),
                        ("d_src1", [NQ, D]), ("d_qpos", [NQ, D]),
                        ("d_val", [128, ELEM]), ("d_idx", [NQ, 64])]:
            dbg[nm] = dt(nm, shp, F32, kind="ExternalOutput")

    groups = [[0, 1, 2, 3], [4, 5, 6, 7]]

    with tile.TileContext(nc) as tc:
        _build_body(nc, tc, ins, out, dbg, groups, debug)
    nc.compile()
    return nc


def _build_body(nc, tc, ins, out, dbg, groups, debug):
    ctx = contextlib.ExitStack()
    consts = ctx.enter_context(tc.tile_pool(name="consts", bufs=1))
    persist = ctx.enter_context(tc.tile_pool(name="persist", bufs=1))
    dpool = ctx.enter_context(tc.tile_pool(name="dpool", bufs=1, space="DRAM"))
    dwork = ctx.enter_context(tc.tile_pool(name="dwork", bufs=2, space="DRAM"))
    tables = [[dpool.tile([TROWS, ELEM], BF16, tag=f"tbl_{i}_{h}",
                          name=f"tbl_{i}_{h}") for h in range(HPC)]
              for i in range(NLAYERS)]

    nc.gpsimd.load_library(library_config.mlp)

    ident = consts.tile([128, 128], F32)
    make_identity(nc, ident[:])

    # const tiles
    cw = consts.tile([128, 6, 32], F32)
    nc.sync.dma_start(out=cw[:], in_=bass.AP(ins["cw"].ap().tensor, 0,
                                             [[0, 128], [32, 6], [1, 32]]))
    locc = consts.tile([128, 4, NQB, 32], F32)
    for ci, nm in enumerate(("ax", "bx", "ay", "by")):
        nc.sync.dma_start(out=locc[:, ci], in_=bass.AP(
            ins[nm].ap().tensor, 0, [[32, 128], [128 * 32, NQB], [1, 32]]))

    # ================= phase 0: query_pos (transposed) =================
    qposT = [persist.tile([128, NQ], F32, tag=f"qposT{m}", name=f"qposT{m}") for m in range(2)]
    with tc.tile_pool(name="ph0", bufs=1) as ph0, \
         tc.tile_pool(name="ph0p", bufs=2, space="PSUM") as ph0p:
        ds = ph0.tile([128, 2], F32)
        nc.sync.dma_start(out=ds[:], in_=ins["ds"][:, :])
        qsT = [ph0.tile([128, NQ], F32, tag=f"qsT{c}", name=f"qsT{c}") for c in range(4)]
        for c in range(4):
            vrow = ph0.tile([128, NQ], F32, tag="vrow")
            nc.sync.dma_start(out=vrow[:], in_=bass.AP(
                ins["refc"].ap().tensor, c * NQ, [[0, 128], [1, NQ]]))
            sg = ph0.tile([128, NQ], F32, tag="sg")
            nc.scalar.activation(out=sg[:], in_=vrow[:], func=AF.Sigmoid)
            nc.scalar.activation(out=qsT[c][:], in_=sg[:], func=AF.Sin,
                                 scale=ds[:, 0:1], bias=ds[:, 1:2])
        rpb1 = ph0.tile([128, 2], F32)
        nc.sync.dma_start(out=rpb1[:], in_=bass.AP(
            ins["rp_b1"].ap().tensor, 0, [[1, 128], [128, 2]]))
        rpb2 = ph0.tile([128, 2], F32)
        nc.sync.dma_start(out=rpb2[:], in_=bass.AP(
            ins["rp_b2"].ap().tensor, 0, [[1, 128], [128, 2]]))
        w1t = ph0.tile([128, 4, D], F32)
        nc.sync.dma_start(out=w1t[:], in_=ins["rp_w1"][:, :].rearrange(
            "(c p) d -> p c d", p=128))
        w2t = ph0.tile([128, 2, D], F32)
        nc.sync.dma_start(out=w2t[:], in_=ins["rp_w2"][:, :].rearrange(
            "(c p) d -> p c d", p=128))
        h1T = [ph0.tile([128, NQ], F32, tag=f"h1T{m}", name=f"h1T{m}") for m in range(2)]
        for m in range(2):
            for qc in range(2):
                pt = ph0p.tile([128, 512], F32, tag="pmlp", name="pmlp")
                for c in range(4):
                    nc.tensor.matmul(out=pt[:],
                                     lhsT=w1t[:, c, m * 128:(m + 1) * 128],
                                     rhs=qsT[c][:, qc * 512:(qc + 1) * 512],
                                     start=(c == 0), stop=(c == 3))
                nc.scalar.activation(out=h1T[m][:, qc * 512:(qc + 1) * 512],
                                     in_=pt[:], func=AF.Relu,
                                     bias=rpb1[:, m:m + 1])
        for m in range(2):
            for qc in range(2):
                pt = ph0p.tile([128, 512], F32, tag="pmlp", name="pmlp")
                for c in range(2):
                    nc.tensor.matmul(out=pt[:],
                                     lhsT=w2t[:, c, m * 128:(m + 1) * 128],
                                     rhs=h1T[c][:, qc * 512:(qc + 1) * 512],
                                     start=(c == 0), stop=(c == 1))
                # qposT = psum + rp_b2 (per-partition)
                nc.vector.tensor_scalar(
                    out=qposT[m][:, qc * 512:(qc + 1) * 512], in0=pt[:],
                    scalar1=rpb2[:, m:m + 1], scalar2=None, op0=AL.add)
    if debug:
        with tc.tile_pool(name="dbg0", bufs=1) as dbg0, \
             tc.tile_pool(name="dbg0p", bufs=2, space="PSUM") as dbg0p:
            qpd = dbg0.tile([128, NQB, D], F32, name="qpd")
            for m in range(2):
                for a in range(NQB):
                    pt = dbg0p.tile([128, 128], F32, tag="ptr", name="ptr")
                    nc.tensor.transpose(out=pt[:],
                                        in_=qposT[m][:, a * 128:(a + 1) * 128],
                                        identity=ident[:])
                    nc.scalar.activation(out=qpd[:, a, m * 128:(m + 1) * 128],
                                         in_=pt[:], func=AF.Copy)
            nc.sync.dma_start(out=_qmaj(dbg["d_qpos"], D), in_=qpd[:])

    # ================= phase 1: value_in + gather tables =================
    vpw = consts.tile([128, NLAYERS, 2, HPC * HD], BF16)
    nc.sync.dma_start(out=vpw[:], in_=ins["vp_w"][:, :, :].rearrange(
        "l (c p) n -> p l c n", p=128))
    ztile = consts.tile([128, HD], BF16)
    nc.vector.memset(ztile[:], 0.0)
    with tc.tile_pool(name="vin", bufs=1) as vin_pool, \
         tc.tile_pool(name="vwork", bufs=3) as vwork, \
         tc.tile_pool(name="vpsum", bufs=4, space="PSUM") as vpsum:
        vin = [vin_pool.tile([128, LEN], BF16, tag=f"vin{c}", name=f"vin{c}") for c in range(2)]
        for c in range(2):
            nc.sync.dma_start(out=vin[c][:],
                              in_=ins["vin"][c * 128:(c + 1) * 128, :])
        for i in range(NLAYERS):
            for s0 in range(0, LEN, 1024):
                nb = min(1024, LEN - s0) // 128
                pt = vpsum.tile([128, 8, HPC * HD], F32, tag="vp", name="vp")
                for t in range(nb):
                    for c in range(2):
                        lhsT = _ap(vin[c], s0 + t, [_p0(vin[c]), [nb, 128]])
                        nc.tensor.matmul(out=pt[:, t], lhsT=lhsT,
                                         rhs=vpw[:, i, c, :],
                                         start=(c == 0), stop=(c == 1))
                st = vwork.tile([128, 8, HPC * HD], BF16, tag="st", name="st")
                nc.scalar.activation(out=st[:, :nb], in_=pt[:, :nb],
                                     func=AF.Identity)
                for h in range(HPC):
                    for cp in range(4):
                        off = (s0 + 1 - cp) * HD + cp * RPC * ELEM
                        eng = (nc.sync, nc.scalar)[(h * 4 + cp) % 2]
                        eng.dma_start(
                            out=bass.AP(tables[i][h].tensor, off,
                                        [[nb * HD, 128], [HD, nb], [1, HD]]),
                            in_=st[:, :nb, h * HD:(h + 1) * HD])
            for h in range(HPC):
                for cp in range(4):
                    if cp == 0:
                        nc.sync.dma_start(
                            out=bass.AP(tables[i][h].tensor, 0,
                                        [[HD, 1], [1, HD]]),
                            in_=ztile[:1, :])
                    lo = LEN + 1
                    n_pad = cp + 3
                    nc.sync.dma_start(
                        out=bass.AP(tables[i][h].tensor,
                                    cp * RPC * ELEM + (lo - cp) * HD,
                                    [[HD, n_pad], [1, HD]]),
                        in_=ztile[:n_pad, :])

    # ================= main-loop pools + weights =================
    lwork = ctx.enter_context(tc.tile_pool(name="lwork", bufs=1))
    big = ctx.enter_context(tc.tile_pool(name="big", bufs=2))
    psum = ctx.enter_context(tc.tile_pool(name="psum", bufs=2, space="PSUM"))
    soaw_w = consts.tile([128, NLAYERS, 2, 96], F32)
    nc.sync.dma_start(out=soaw_w[:], in_=ins["soaw_w"][:, :, :].rearrange(
        "l (c p) n -> p l c n", p=128))
    soaw_b = consts.tile([128, NLAYERS, 96], F32)
    nc.sync.dma_start(out=soaw_b[:], in_=bass.AP(
        ins["soaw_b"].ap().tensor, 0, [[0, 128], [96, NLAYERS], [1, 96]]))
    opw = consts.tile([68, NLAYERS, D], F32)
    nc.sync.dma_start(out=opw[:], in_=ins["opw_aug"][:, :, :].rearrange(
        "l p n -> p l n"))
    f1bT = consts.tile([128, NLAYERS, 8], F32)
    nc.sync.dma_start(out=f1bT[:], in_=bass.AP(
        ins["f1_b"].ap().tensor, 0, [[1, 128], [DFF, NLAYERS], [128, 8]]))
    fwins = (ins["f1_w"], ins["f2_w"])
    fbins = {nm: ins[nm] for nm in ("f2_b", "n1_s", "n1_b", "n2_s", "n2_b")}

    src = persist.tile([128, NQB, D], F32)
    nc.sync.dma_start(out=src[:], in_=ins["tgts"][:, :].rearrange(
        "(a p) d -> p a d", p=128))
    eps = consts.tile([128, 1], F32)
    nc.vector.memset(eps[:], 1e-5)

    for li in range(NLAYERS):
        _layer(nc, li, src, qposT, soaw_w, soaw_b, opw, fwins, f1bT, fbins,
               cw, locc, ident, eps, tables[li], dwork,
               groups, lwork, big, psum, dbg, debug and li == 0)

    nc.sync.dma_start(out=_qmaj(out, D), in_=src[:])
    if debug:
        nc.gpsimd.dma_start(out=dbg["d_val"][:, :],
                            in_=bass.AP(tables[0][0].tensor, 0,
                                        [[ELEM, 128], [1, ELEM]]))
    ctx.close()


def _layer(nc, li, src, qposT, soaw_w, soaw_b, opw, fwins, f1bT, fbins,
           cw, locc, ident, eps, tbls, dwork, groups,
           lwork, big, psum, dbg, dump):
    P4 = [128, NQB, 2, 16]
    # per-layer FFN weights + bias rows
    f1w = lwork.tile([128, 2, DFF], BF16, tag="f1w", name="f1w")
    nc.sync.dma_start(out=f1w[:], in_=fwins[0][li, :, :].rearrange(
        "(c p) n -> p c n", p=128))
    f2w = lwork.tile([128, 8, D], BF16, tag="f2w", name="f2w")
    nc.sync.dma_start(out=f2w[:], in_=fwins[1][li, :, :].rearrange(
        "(c p) n -> p c n", p=128))
    fb = {}
    for nm, dr in fbins.items():
        fb[nm] = lwork.tile([128, D], F32, tag=f"fb_{nm}", name=f"fb_{nm}")
        nc.sync.dma_start(out=fb[nm][:], in_=bass.AP(
            dr.ap().tensor, li * D, [[0, 128], [1, D]]))

    def T(tag, shape=None, dtp=F32):
        return lwork.tile(shape or P4, dtp, tag=tag, name=tag)

    def tt(o, i0, i1, op):
        nc.any.tensor_tensor(out=o, in0=i0, in1=i1, op=op)

    # ---- qT = srcT + qposT ----
    qT = [T(f"qT{m}", [128, NQ]) for m in range(2)]
    for a in range(NQB):
        for m in range(2):
            pt = psum.tile([128, 128], F32, tag="ptr", name="ptr")
            nc.tensor.transpose(out=pt[:], in_=src[:, a, m * 128:(m + 1) * 128],
                                identity=ident[:])
            nc.scalar.activation(out=qT[m][:, a * 128:(a + 1) * 128],
                                 in_=pt[:], func=AF.Copy)
    for m in range(2):
        tt(qT[m][:], qT[m][:], qposT[m][:], AL.add)

    # ---- so/aw ----
    soaw = T("soaw", [128, NQB, 96])
    for a in range(NQB):
        pt = psum.tile([128, 96], F32, tag="p256", name="p256")
        for m in range(2):
            nc.tensor.matmul(out=pt[:], lhsT=qT[m][:, a * 128:(a + 1) * 128],
                             rhs=soaw_w[:, li, m, :], start=(m == 0),
                             stop=(m == 1))
        nc.vector.tensor_tensor(out=soaw[:, a], in0=pt[:], in1=soaw_b[:, li],
                                op=AL.add)
    if dump:
        nc.sync.dma_start(out=_qmaj(dbg["d_so"], 64), in_=_ap(
            soaw, 0, [_p0(soaw), [96, NQB], [1, 64]]))

    # ---- softmax over 16 per (q, h) ----
    aw = T("aw")
    mx = T("mx", [128, NQB, 2])
    awl = _ap(soaw, 64, [_p0(soaw), [96, NQB], [16, 2], [1, 16]])
    nc.vector.tensor_reduce(out=mx[:], in_=awl, axis=AX.X, op=AL.max)
    tt(aw[:], awl, _ap(mx, 0, [_p0(mx), [2, NQB], [1, 2], [0, 16]]),
       AL.subtract)
    nc.scalar.activation(out=aw[:], in_=aw[:], func=AF.Exp)
    sm = T("sm", [128, NQB, 2])
    nc.vector.tensor_reduce(out=sm[:], in_=aw[:], axis=AX.X, op=AL.add)
    nc.vector.reciprocal(out=sm[:], in_=sm[:])
    tt(aw[:], aw[:], _ap(sm, 0, [_p0(sm), [2, NQB], [1, 2], [0, 16]]), AL.mult)
    if dump:
        nc.sync.dma_start(out=_qmaj(dbg["d_aw"], 32), in_=_ap(
            aw, 0, [_p0(aw), [32, NQB], [1, 32]]))

    # ---- pixel coords ----
    sox = _ap(soaw, 0, [_p0(soaw), [96, NQB], [32, 2], [2, 16]])
    soy = _ap(soaw, 1, [_p0(soaw), [96, NQB], [32, 2], [2, 16]])

    def lc(ci):
        return _ap(locc, ci * NQB * 32,
                   [_p0(locc), [32, NQB], [0, 2], [1, 16]])

    def cwv(r):
        return _ap(cw, r * 32, [_p0(cw), [0, NQB], [16, 2], [1, 16]])

    x, y = T("x"), T("y")
    tt(x[:], sox, lc(1), AL.mult)
    tt(x[:], x[:], lc(0), AL.add)
    tt(y[:], soy, lc(3), AL.mult)
    tt(y[:], y[:], lc(2), AL.add)
    if dump:
        nc.sync.dma_start(out=_qmaj(dbg["d_x"], 32), in_=_ap(
            x, 0, [_p0(x), [32, NQB], [1, 32]]))
        nc.sync.dma_start(out=_qmaj(dbg["d_y"], 32), in_=_ap(
            y, 0, [_p0(y), [32, NQB], [1, 32]]))

    ti = T("ti", P4, I32)
    tf = T("tf")
    fx, fy = T("fx"), T("fy")

    def floor_(dst, xin):
        nc.vector.tensor_copy(out=ti[:], in_=xin)
        nc.vector.tensor_copy(out=tf[:], in_=ti[:])
        nc.vector.tensor_tensor(out=dst[:], in0=tf[:], in1=xin, op=AL.is_gt)
        nc.vector.tensor_tensor(out=dst[:], in0=tf[:], in1=dst[:],
                                op=AL.subtract)

    floor_(fx, x[:])
    floor_(fy, y[:])
    lx, ly = T("lx"), T("ly")
    tt(lx[:], x[:], fx[:], AL.subtract)
    tt(ly[:], y[:], fy[:], AL.subtract)

    t1 = T("t1")
    ix0, ix1, iy0, iy1 = T("iy0"), T("iy1"), T("iy2"), T("iy3")
    for (dst, f, hi) in ((ix0, fx, 3), (iy0, fy, 4)):
        nc.any.tensor_scalar(out=dst[:], in0=f[:], scalar1=0.0, scalar2=None,
                             op0=AL.is_ge)
        tt(t1[:], f[:], cwv(hi), AL.is_le)
        tt(dst[:], dst[:], t1[:], AL.mult)
    for (dst, f, hi) in ((ix1, fx, 5), (iy1, fy, -1)):
        nc.any.tensor_scalar(out=dst[:], in0=f[:], scalar1=-1.0, scalar2=None,
                             op0=AL.is_ge)
        if hi >= 0:
            tt(t1[:], f[:], cwv(hi), AL.is_le)          # fx <= W-2
        else:
            nc.any.tensor_scalar(out=t1[:], in0=f[:], scalar1=1.0,
                                 scalar2=None, op0=AL.add)
            tt(t1[:], t1[:], cwv(4), AL.is_le)          # fy+1 <= H-1
        tt(dst[:], dst[:], t1[:], AL.mult)

    wy0, wy1, wx0, wx1 = T("wy0"), T("wy1"), T("ix0"), T("ix1")
    nc.any.tensor_scalar(out=wy0[:], in0=ly[:], scalar1=-1.0, scalar2=-1.0,
                         op0=AL.add, op1=AL.mult)       # (ly-1)*-1
    tt(wy0[:], wy0[:], iy0[:], AL.mult)
    tt(wy0[:], wy0[:], aw[:], AL.mult)
    tt(wy1[:], ly[:], iy1[:], AL.mult)
    tt(wy1[:], wy1[:], aw[:], AL.mult)
    nc.any.tensor_scalar(out=wx0[:], in0=lx[:], scalar1=-1.0, scalar2=-1.0,
                         op0=AL.add, op1=AL.mult)
    tt(wx0[:], wx0[:], ix0[:], AL.mult)
    tt(wx1[:], lx[:], ix1[:], AL.mult)

    wall = T("wall", [128, NQB, 2, 16, 2, 2], BF16)
    for r, wy in ((0, wy0), (1, wy1)):
        for c, wx in ((0, wx0), (1, wx1)):
            tt(t1[:], wy[:], wx[:], AL.mult)
            nc.vector.tensor_copy(out=_ap(
                wall, r * 2 + c,
                [_p0(wall), [128, NQB], [64, 2], [4, 16]]), in_=t1[:])
    if dump:
        nc.gpsimd.dma_start(out=_qmaj(dbg["d_w"], 128), in_=_ap(
            wall, 0, [_p0(wall), [128, NQB], [1, 128]]))

    samp = T("samp", [128, NQB, 68])
    nc.vector.tensor_reduce(out=_ap(samp, 64, [_p0(samp), [68, NQB], [1, 2]]),
                            in_=_ap(wall, 0, [_p0(wall), [128, NQB], [64, 2],
                                              [1, 64]]),
                            axis=AX.X, op=AL.add)
    nc.vector.memset(samp[:, :, 66:68], 1.0)

    # ---- flat index -> table rows ----
    s0, s1 = T("x"), T("y")  # reuse dead slots
    tt(s0[:], fy[:], cwv(0), AL.mult)
    tt(s0[:], s0[:], fx[:], AL.add)
    tt(s0[:], s0[:], cwv(2), AL.add)
    tt(s1[:], s0[:], cwv(0), AL.add)
    if dump:
        nc.sync.dma_start(out=_qmaj(dbg["d_s0"], 32), in_=_ap(
            s0, 0, [_p0(s0), [32, NQB], [1, 32]]))
    idxe = T("idxe", [128, NQB, 2, 16, 2])
    for r, s in ((0, s0), (1, s1)):
        nc.any.tensor_scalar(out=s[:], in0=s[:], scalar1=0.0, scalar2=SMAX,
                             op0=AL.max, op1=AL.min)
        nc.any.tensor_scalar(out=t1[:], in0=s[:], scalar1=0.25, scalar2=-0.375,
                             op0=AL.mult, op1=AL.add)
        nc.vector.tensor_copy(out=ti[:], in_=t1[:])
        nc.vector.tensor_copy(out=t1[:], in_=ti[:])
        # c = s - 4*floor4 in [0,3]; idx = c*RPC + floor4  (all exact in f32)
        nc.vector.scalar_tensor_tensor(out=tf[:], in0=t1[:], scalar=-4.0,
                                       in1=s[:], op0=AL.mult, op1=AL.add)
        nc.vector.scalar_tensor_tensor(
            out=_ap(idxe, r, [_p0(idxe), [64, NQB], [32, 2], [2, 16]]),
            in0=tf[:], scalar=float(RPC), in1=t1[:],
            op0=AL.mult, op1=AL.add)
    if dump:
        nc.sync.dma_start(out=_qmaj(dbg["d_idx"], 64), in_=_ap(
            idxe, 0, [_p0(idxe), [64, NQB], [1, 64]]))

    # ---- idx wrap bridge ----
    idxT = T("qT0", [128, NQ])
    for a in range(NQB):
        pt = psum.tile([128, 128], F32, tag="ptr", name="ptr")
        nc.tensor.transpose(out=pt[:64, :],
                            in_=_ap(idxe, a * 64, [_p0(idxe), [1, 64]]),
                            identity=ident[:])
        nc.scalar.activation(out=idxT[:64, a * 128:(a + 1) * 128],
                             in_=pt[:64, :], func=AF.Copy)
    st16 = T("st16", [16, NQB, 2, 32, 8], I16)
    for aq in range(NQB):
        for a8 in range(8):
            pt = psum.tile([128, 128], F32, tag="ptr", name="ptr")
            nc.tensor.transpose(
                out=pt[:16, :64],
                in_=idxT[:64, aq * 128 + a8 * 16:aq * 128 + a8 * 16 + 16],
                identity=ident[:64, :64])
            nc.vector.tensor_copy(out=_ap(
                st16, aq * 512 + a8, [_p0(st16), [256, 2], [8, 32]]),
                in_=pt[:16, :64])
    idx_dram = dwork.tile([16, NQB * HPC * KP * 8], I16, tag="idxd",
                          name="idxd")
    nc.sync.dma_start(out=idx_dram[:, :], in_=st16[:].rearrange(
        "p a h k e -> p (a h k e)"))

    # ---- gather + weighted sum ----
    for a in range(NQB):
        for h in range(HPC):
            idx16 = big.tile([128, 256], I16, tag="idx16", name="idx16")
            nc.sync.dma_start(out=idx16[:], in_=_ap(
                idx_dram, a * 512 + h * 256,
                [[0, 8], [4096, 16], [1, 256]]))
            g = big.tile([128, KP, ELEM], BF16, tag="g", name="g")
            if SKIP_GATHER:
                nc.vector.memset(g[:], 0.25)
            else:
                nc.gpsimd.dma_gather(
                    out_ap=g[:], in_ap=tbls[h][:, :], idxs_ap=idx16[:],
                    num_idxs=128 * KP, num_idxs_reg=128 * KP, elem_size=ELEM,
                    single_packet=False, queue_num=(a * HPC + h) % 4)
            m = big.tile([128, KP, 64], BF16, tag="m", name="m")
            nc.any.tensor_tensor(
                out=m[:],
                in0=_ap(g, 0, [_p0(g), [ELEM, KP], [32, 2], [1, 32]]),
                in1=_ap(wall, a * 128 + h * 64,
                        [_p0(wall), [2, KP], [1, 2], [0, 32]]),
                op=AL.mult)
            s = big.tile([128, KP, 32], BF16, tag="s", name="s")
            nc.any.tensor_tensor(out=s[:], in0=m[:, :, 0:32],
                                 in1=m[:, :, 32:64], op=AL.add)
            nc.vector.tensor_reduce(
                out=samp[:, a, h * 32:(h + 1) * 32],
                in_=_ap(s, 0, [_p0(s), [1, 32], [32, KP]]),
                axis=AX.X, op=AL.add)
    if dump:
        nc.sync.dma_start(out=_qmaj(dbg["d_samp"], 68), in_=samp[:])

    # ---- sampledT + op projection + AllReduce ----
    sampT = T("qT1", [68, NQ])
    for a in range(NQB):
        pt = psum.tile([128, 128], F32, tag="ptr", name="ptr")
        nc.tensor.transpose(out=pt[:68, :], in_=samp[:, a, :],
                            identity=ident[:])
        nc.scalar.activation(out=sampT[:, a * 128:(a + 1) * 128],
                             in_=pt[:68, :], func=AF.Copy)
    attn = T("attn", [128, NQB, D])
    for a in range(NQB):
        pt = psum.tile([128, D], F32, tag="p256", name="p256")
        nc.tensor.matmul(out=pt[:], lhsT=sampT[:, a * 128:(a + 1) * 128],
                         rhs=opw[:, li, :], start=True, stop=True)
        nc.scalar.activation(out=attn[:, a], in_=pt[:], func=AF.Copy)
    if not SKIP_CC:
        cc_in = dwork.tile([NQ, D], F32, tag="cc_in", name="cc_in")
        cc_out = dwork.tile([NQ, D], F32, tag="cc_out", name="cc_out")
        nc.sync.dma_start(out=_qmaj(cc_in, D), in_=attn[:])
        nc.gpsimd.collective_compute(
            "AllReduce", AL.add, replica_groups=groups,
            ins=[cc_in[:].opt()], outs=[cc_out[:].opt()])
        nc.sync.dma_start(out=attn[:], in_=_qmaj(cc_out, D))
    if dump:
        nc.sync.dma_start(out=_qmaj(dbg["d_attn"], D), in_=attn[:])

    _layernorm(nc, src, attn, fb["n1_s"], fb["n1_b"], eps, lwork, src)
    if dump:
        nc.sync.dma_start(out=_qmaj(dbg["d_src1"], D), in_=src[:])

    # ---- FFN ----
    s1T = [T(f"s1T{m}", [128, NQ], BF16) for m in range(2)]
    for a in range(NQB):
        for m in range(2):
            pt = psum.tile([128, 128], F32, tag="ptr", name="ptr")
            nc.tensor.transpose(out=pt[:], in_=src[:, a, m * 128:(m + 1) * 128],
                                identity=ident[:])
            nc.scalar.activation(out=s1T[m][:, a * 128:(a + 1) * 128],
                                 in_=pt[:], func=AF.Copy)
    hT = [T(f"hT{m8}", [128, NQ], BF16) for m8 in range(8)]
    for m8 in range(8):
        for qc in range(2):
            pt = psum.tile([128, 512], F32, tag="p512", name="p512")
            for m in range(2):
                nc.tensor.matmul(out=pt[:],
                                 lhsT=f1w[:, m, m8 * 128:(m8 + 1) * 128],
                                 rhs=s1T[m][:, qc * 512:(qc + 1) * 512],
                                 start=(m == 0), stop=(m == 1))
            nc.scalar.activation(out=hT[m8][:, qc * 512:(qc + 1) * 512],
                                 in_=pt[:], func=AF.Relu,
                                 bias=f1bT[:, li, m8:m8 + 1])
    ffn = T("attn", [128, NQB, D])
    for a in range(NQB):
        pt = psum.tile([128, D], F32, tag="p256", name="p256")
        for m8 in range(8):
            nc.tensor.matmul(out=pt[:], lhsT=hT[m8][:, a * 128:(a + 1) * 128],
                             rhs=f2w[:, m8, :], start=(m8 == 0),
                             stop=(m8 == 7))
        nc.vector.tensor_tensor(out=ffn[:, a], in0=pt[:], in1=fb["f2_b"][:],
                                op=AL.add)
    _layernorm(nc, src, ffn, fb["n2_s"], fb["n2_b"], eps, lwork, src)


def _layernorm(nc, src, delta, gamma_t, beta_t, eps, lwork, out_tile):
    # delta <- src + delta;  out_tile <- LN(delta)*gamma + beta
    nc.any.tensor_tensor(out=delta[:], in0=src[:], in1=delta[:], op=AL.add)
    stats = lwork.tile([128, NQB, 6], F32, tag="ln_st", name="ln_st")
    mv = lwork.tile([128, NQB, 2], F32, tag="ln_mv", name="ln_mv")
    for a in range(NQB):
        nc.vector.bn_stats(out=stats[:, a], in_=delta[:, a])
        nc.vector.bn_aggr(out=mv[:, a], in_=stats[:, a])
    rstd = lwork.tile([128, NQB], F32, tag="ln_rs", name="ln_rs")
    nc.scalar.activation(out=rstd[:], in_=_ap(mv, 1, [_p0(mv), [2, NQB]]),
                         func=AF.Sqrt, bias=eps[:, 0:1])
    nc.vector.reciprocal(out=rstd[:], in_=rstd[:])
    nmr = lwork.tile([128, NQB], F32, tag="ln_nm", name="ln_nm")
    nc.vector.tensor_tensor(out=nmr[:], in0=_ap(mv, 0, [_p0(mv), [2, NQB]]),
                            in1=rstd[:], op=AL.mult)
    nc.any.tensor_scalar(out=nmr[:], in0=nmr[:], scalar1=-1.0, scalar2=None,
                         op0=AL.mult)
    for a in range(NQB):
        nc.vector.tensor_scalar(out=delta[:, a], in0=delta[:, a],
                                scalar1=rstd[:, a:a + 1],
                                scalar2=nmr[:, a:a + 1],
                                op0=AL.mult, op1=AL.add)
    g = _ap(gamma_t, 0, [_p0(gamma_t), [0, NQB], [1, D]])
    b = _ap(beta_t, 0, [_p0(beta_t), [0, NQB], [1, D]])
    nc.any.tensor_tensor(out=delta[:], in0=delta[:], in1=g, op=AL.mult)
    nc.any.tensor_tensor(out=out_tile[:], in0=delta[:], in1=b, op=AL.add)


# ======================= host side =======================

def _expand32(v_ql):
    return np.ascontiguousarray(
        np.broadcast_to(v_ql[:, None, :, None], (NQ, HPC, NL, NP))
        .reshape(NQ, 32)).astype(np.float32)


def _host_inputs(inputs):
    import ml_dtypes
    tgts = np.asarray(inputs["tgts"], np.float32)
    refp = np.asarray(inputs["reference_points"], np.float32)
    masks = [np.asarray(inputs[f"mask{l}"]) for l in range(NL)]
    vrs = []
    for m in masks:
        H, W = m.shape[1], m.shape[2]
        vh = (~m[:, :, 0]).sum(1).astype(np.float32) / H
        vw = (~m[:, 0, :]).sum(1).astype(np.float32) / W
        vrs.append(np.stack([vw, vh], -1))
    vr = np.stack(vrs, 1)

    i = np.arange(128, dtype=np.float64)
    dim_t = 10000.0 ** (2 * np.floor(i / 2) / 128)
    ds = np.stack([2 * math.pi / dim_t,
                   np.where(np.arange(128) % 2 == 0, 0.0, math.pi / 2)],
                  -1).astype(np.float32)

    cw = np.zeros((6, 32), np.float32)
    for h2 in range(HPC):
        for l in range(NL):
            sl = slice(h2 * 16 + l * NP, h2 * 16 + (l + 1) * NP)
            cw[0, sl] = LVL_W[l]
            cw[1, sl] = LVL_H[l]
            cw[2, sl] = LVL_BASE[l]
            cw[3, sl] = LVL_W[l] - 1
            cw[4, sl] = LVL_H[l] - 1
            cw[5, sl] = LVL_W[l] - 2

    so_w = np.asarray(inputs["so_w"], np.float32)
    so_b = np.asarray(inputs["so_b"], np.float32)
    aw_w = np.asarray(inputs["aw_w"], np.float32)
    aw_b = np.asarray(inputs["aw_b"], np.float32)
    vp_w = np.asarray(inputs["vp_w"], np.float32)
    vp_b = np.asarray(inputs["vp_b"], np.float32)
    op_w = np.asarray(inputs["op_w"], np.float32)
    op_b = np.asarray(inputs["op_b"], np.float32)
    Wv = np.array(LVL_W, np.float32)[None]
    Hv = np.array(LVL_H, np.float32)[None]

    lemb = np.asarray(inputs["level_embed"], np.float32)
    vin_b = []
    for b in range(BS):
        parts = []
        for l in range(NL):
            parts.append(
                np.asarray(inputs[f"mem{l}"], np.float32)[b].reshape(D, -1)
                + np.asarray(inputs[f"pm{l}"], np.float32)[b].reshape(D, -1)
                + lemb[l][:, None])
        vin_b.append(np.ascontiguousarray(
            np.concatenate(parts, 1)).astype(ml_dtypes.bfloat16))

    in_maps = []
    for core in range(NCORES):
        b, hg = core // 4, core % 4
        h0 = 2 * hg
        m = {}
        m["tgts"] = tgts[b]
        m["refc"] = np.ascontiguousarray(refp[b].T[[1, 0, 2, 3]])
        m["vin"] = vin_b[b]
        ri = refp[b][:, None, :] * np.concatenate([vr[b], vr[b]], -1)[None]
        m["ax"] = _expand32(ri[:, :, 0] * Wv - 0.5)
        m["bx"] = _expand32(ri[:, :, 2] * Wv * (0.5 / NP))
        m["ay"] = _expand32(ri[:, :, 1] * Hv - 0.5)
        m["by"] = _expand32(ri[:, :, 3] * Hv * (0.5 / NP))
        m["cw"] = cw
        m["ds"] = ds
        m["rp_w1"] = np.asarray(inputs["rp_w1"], np.float32)
        m["rp_b1"] = np.asarray(inputs["rp_b1"], np.float32)
        m["rp_w2"] = np.asarray(inputs["rp_w2"], np.float32)
        m["rp_b2"] = np.asarray(inputs["rp_b2"], np.float32)
        so_r = so_w.reshape(NLAYERS, D, NH, NL, NP, 2)
        aw_r = aw_w.reshape(NLAYERS, D, NH, NL, NP)
        sob_r = so_b.reshape(NLAYERS, NH, NL, NP, 2)
        awb_r = aw_b.reshape(NLAYERS, NH, NL, NP)
        m["soaw_w"] = np.ascontiguousarray(np.concatenate([
            so_r[:, :, h0:h0 + 2].reshape(NLAYERS, D, 64),
            aw_r[:, :, h0:h0 + 2].reshape(NLAYERS, D, 32)], -1))
        m["soaw_b"] = np.ascontiguousarray(np.concatenate([
            sob_r[:, h0:h0 + 2].reshape(NLAYERS, 64),
            awb_r[:, h0:h0 + 2].reshape(NLAYERS, 32)], -1))
        m["vp_w"] = np.ascontiguousarray(vp_w[:, :, h0 * HD:(h0 + 2) * HD]).astype(ml_dtypes.bfloat16)
        opa = np.zeros((NLAYERS, 68, D), np.float32)
        for lii in range(NLAYERS):
            opa[lii, 0:64] = op_w[lii, h0 * HD:(h0 + 2) * HD]
            for hh in range(HPC):
                sl = slice((h0 + hh) * HD, (h0 + hh + 1) * HD)
                opa[lii, 64 + hh] = vp_b[lii, sl] @ op_w[lii, sl]
            opa[lii, 66] = op_b[lii] / 4.0
        m["opw_aug"] = opa
        m["f1_w"] = np.asarray(inputs["f1_w"], np.float32).astype(
            ml_dtypes.bfloat16)
        m["f2_w"] = np.asarray(inputs["f2_w"], np.float32).astype(
            ml_dtypes.bfloat16)
        m["f1_b"] = np.asarray(inputs["f1_b"], np.float32)
        m["f2_b"] = np.asarray(inputs["f2_b"], np.float32)
        for nm in ("n1_s", "n1_b", "n2_s", "n2_b"):
            m[nm] = np.asarray(inputs[nm], np.float32)
        in_maps.append(m)
    return in_maps


def kernel(**inputs):
    if "nc" not in _CACHE:
        _CACHE["nc"] = build_nc(debug=False)
    nc = _CACHE["nc"]
    in_maps = _host_inputs(inputs)
    res = run_bass_kernel_spmd(nc, in_maps, list(range(NCORES)))
    return np.stack([res.results[0]["out"],
                     res.results[4]["out"]]).astype(np.float32)



# revision 12
# speedup vs baseline: 1.5221x; 1.2481x over previous
"""Trainium2 Bass kernel for CISTransformerDecoder (deformable-attention decoder).

Sharding: 8 cores = 2 batches x 4 head-groups (2 heads each).
Each core materializes per-(layer,head) bf16 gather tables (4 shifted copies,
256B units), runs all 1024 queries for its 2 heads, exchanges partial
attention outputs via 4-core AllReduce, and replicates LN/FFN.
"""
import sys, math, contextlib
sys.path.insert(0, "/opt/trn_rl_repo")
import numpy as np

import concourse.bass as bass
import concourse.bacc as bacc
import concourse.tile as tile
from concourse import mybir
from concourse import library_config
from concourse.bass_utils import run_bass_kernel_spmd
from concourse.masks import make_identity

F32, BF16, I32, I16 = (mybir.dt.float32, mybir.dt.bfloat16, mybir.dt.int32,
                       mybir.dt.int16)
AF = mybir.ActivationFunctionType
AL = mybir.AluOpType
AX = mybir.AxisListType

SHAPES = ((128, 128), (64, 64), (32, 32), (16, 16))
NL, NH, NP, D, HD = 4, 8, 4, 256, 32
NLAYERS, DFF, BS, NQ = 6, 1024, 2, 1024
LEN = sum(h * w for h, w in SHAPES)        # 21760
NCORES, HPC = 8, 2
NQB = NQ // 128                             # 8
KP = NL * NP * 2                            # 32 gather units per (q, head)
RPC = 5441                                  # table rows per copy
TROWS = 4 * RPC                             # 21764
ELEM = 128                                  # bf16 elems per unit (256B)
SMAX = float(LEN)
LVL_W = [w for (h, w) in SHAPES]
LVL_H = [h for (h, w) in SHAPES]
LVL_BASE = [1, 1 + 16384, 1 + 16384 + 4096, 1 + 16384 + 4096 + 1024]

_CACHE = {}
SKIP_GATHER = False
SKIP_CC = False


def _ap(t, off, dims):
    return bass.AP(t.tensor, t.offset + off, dims)


def _p0(t):
    return list(t.ap[0])



def _qmaj(dram_t, n):
    # DRAM AP for [(a p), n] laid out query-major, enumerated as (p, a, n)
    t = dram_t if isinstance(dram_t, bass.AP) else dram_t.ap()
    return bass.AP(t.tensor, t.offset, [[n, 128], [128 * n, NQB], [1, n]])

def build_nc(debug=False):
    nc = bacc.Bacc("TRN2", target_bir_lowering=False, debug=False,
                   num_devices=NCORES, num_swdge_queues=4)
    dt = nc.dram_tensor
    ins = {}
    ins["tgts"] = dt("tgts", [NQ, D], F32, kind="ExternalInput")
    ins["refc"] = dt("refc", [4, NQ], F32, kind="ExternalInput")
    ins["vin"] = dt("vin", [D, LEN], BF16, kind="ExternalInput")
    for nm in ("ax", "bx", "ay", "by"):
        ins[nm] = dt(nm, [NQ, 32], F32, kind="ExternalInput")
    ins["cw"] = dt("cw", [6, 32], F32, kind="ExternalInput")
    ins["ds"] = dt("ds", [128, 2], F32, kind="ExternalInput")
    ins["rp_w1"] = dt("rp_w1", [512, D], F32, kind="ExternalInput")
    ins["rp_b1"] = dt("rp_b1", [D], F32, kind="ExternalInput")
    ins["rp_w2"] = dt("rp_w2", [D, D], F32, kind="ExternalInput")
    ins["rp_b2"] = dt("rp_b2", [D], F32, kind="ExternalInput")
    ins["soaw_w"] = dt("soaw_w", [NLAYERS, D, 96], F32, kind="ExternalInput")
    ins["soaw_b"] = dt("soaw_b", [NLAYERS, 96], F32, kind="ExternalInput")
    ins["vp_w"] = dt("vp_w", [NLAYERS, D, HPC * HD], BF16, kind="ExternalInput")
    ins["opw_aug"] = dt("opw_aug", [NLAYERS, 68, D], F32, kind="ExternalInput")
    ins["f1_w"] = dt("f1_w", [NLAYERS, D, DFF], BF16, kind="ExternalInput")
    ins["f1_b"] = dt("f1_b", [NLAYERS, DFF], F32, kind="ExternalInput")
    ins["f2_w"] = dt("f2_w", [NLAYERS, DFF, D], BF16, kind="ExternalInput")
    ins["f2_b"] = dt("f2_b", [NLAYERS, D], F32, kind="ExternalInput")
    for nm in ("n1_s", "n1_b", "n2_s", "n2_b"):
        ins[nm] = dt(nm, [NLAYERS, D], F32, kind="ExternalInput")

    out = dt("out", [NQ, D], F32, kind="ExternalOutput")
    dbg = {}
    if debug:
        for nm, shp in [("d_so", [NQ, 64]), ("d_aw", [NQ, 32]),
                        ("d_x", [NQ, 32]), ("d_y", [NQ, 32]),
                        ("d_s0", [NQ, 32]), ("d_w", [NQ, 128]),
                        ("d_samp", [NQ, 68]), ("d_attn", [NQ, D]),
                        ("d_src1", [NQ, D]), ("d_qpos", [NQ, D]),
                        ("d_val", [128, ELEM]), ("d_idx", [NQ, 64])]:
            dbg[nm] = dt(nm, shp, F32, kind="ExternalOutput")

    groups = [[0, 1, 2, 3], [4, 5, 6, 7]]

    with tile.TileContext(nc) as tc:
        _build_body(nc, tc, ins, out, dbg, groups, debug)
    nc.compile()
    return nc


def _build_body(nc, tc, ins, out, dbg, groups, debug):
    ctx = contextlib.ExitStack()
    consts = ctx.enter_context(tc.tile_pool(name="consts", bufs=1))
    persist = ctx.enter_context(tc.tile_pool(name="persist", bufs=1))
    dpool = ctx.enter_context(tc.tile_pool(name="dpool", bufs=1, space="DRAM"))
    dwork = ctx.enter_context(tc.tile_pool(name="dwork", bufs=2, space="DRAM"))
    tables = [[dpool.tile([TROWS, ELEM], BF16, tag=f"tbl_{i}_{h}",
                          name=f"tbl_{i}_{h}") for h in range(HPC)]
              for i in range(NLAYERS)]

    nc.gpsimd.load_library(library_config.mlp)

    ident = consts.tile([128, 128], F32)
    make_identity(nc, ident[:])

    # const tiles
    cw = consts.tile([128, 6, 32], F32)
    nc.sync.dma_start(out=cw[:], in_=bass.AP(ins["cw"].ap().tensor, 0,
                                             [[0, 128], [32, 6], [1, 32]]))
    locc = consts.tile([128, 4, NQB, 32], F32)
    for ci, nm in enumerate(("ax", "bx", "ay", "by")):
        nc.sync.dma_start(out=locc[:, ci], in_=bass.AP(
            ins[nm].ap().tensor, 0, [[32, 128], [128 * 32, NQB], [1, 32]]))

    # ================= phase 0: query_pos (transposed) =================
    qposT = [persist.tile([128, NQ], F32, tag=f"qposT{m}", name=f"qposT{m}") for m in range(2)]
    with tc.tile_pool(name="ph0", bufs=1) as ph0, \
         tc.tile_pool(name="ph0p", bufs=2, space="PSUM") as ph0p:
        ds = ph0.tile([128, 2], F32)
        nc.sync.dma_start(out=ds[:], in_=ins["ds"][:, :])
        qsT = [ph0.tile([128, NQ], F32, tag=f"qsT{c}", name=f"qsT{c}") for c in range(4)]
        for c in range(4):
            vrow = ph0.tile([128, NQ], F32, tag="vrow")
            nc.sync.dma_start(out=vrow[:], in_=bass.AP(
                ins["refc"].ap().tensor, c * NQ, [[0, 128], [1, NQ]]))
            sg = ph0.tile([128, NQ], F32, tag="sg")
            nc.scalar.activation(out=sg[:], in_=vrow[:], func=AF.Sigmoid)
            nc.scalar.activation(out=qsT[c][:], in_=sg[:], func=AF.Sin,
                                 scale=ds[:, 0:1], bias=ds[:, 1:2])
        rpb1 = ph0.tile([128, 2], F32)
        nc.sync.dma_start(out=rpb1[:], in_=bass.AP(
            ins["rp_b1"].ap().tensor, 0, [[1, 128], [128, 2]]))
        rpb2 = ph0.tile([128, 2], F32)
        nc.sync.dma_start(out=rpb2[:], in_=bass.AP(
            ins["rp_b2"].ap().tensor, 0, [[1, 128], [128, 2]]))
        w1t = ph0.tile([128, 4, D], F32)
        nc.sync.dma_start(out=w1t[:], in_=ins["rp_w1"][:, :].rearrange(
            "(c p) d -> p c d", p=128))
        w2t = ph0.tile([128, 2, D], F32)
        nc.sync.dma_start(out=w2t[:], in_=ins["rp_w2"][:, :].rearrange(
            "(c p) d -> p c d", p=128))
        h1T = [ph0.tile([128, NQ], F32, tag=f"h1T{m}", name=f"h1T{m}") for m in range(2)]
        for m in range(2):
            for qc in range(2):
                pt = ph0p.tile([128, 512], F32, tag="pmlp", name="pmlp")
                for c in range(4):
                    nc.tensor.matmul(out=pt[:],
                                     lhsT=w1t[:, c, m * 128:(m + 1) * 128],
                                     rhs=qsT[c][:, qc * 512:(qc + 1) * 512],
                                     start=(c == 0), stop=(c == 3))
                nc.scalar.activation(out=h1T[m][:, qc * 512:(qc + 1) * 512],
                                     in_=pt[:], func=AF.Relu,
                                     bias=rpb1[:, m:m + 1])
        for m in range(2):
            for qc in range(2):
                pt = ph0p.tile([128, 512], F32, tag="pmlp", name="pmlp")
                for c in range(2):
                    nc.tensor.matmul(out=pt[:],
                                     lhsT=w2t[:, c, m * 128:(m + 1) * 128],
                                     rhs=h1T[c][:, qc * 512:(qc + 1) * 512],
                                     start=(c == 0), stop=(c == 1))
                # qposT = psum + rp_b2 (per-partition)
                nc.vector.tensor_scalar(
                    out=qposT[m][:, qc * 512:(qc + 1) * 512], in0=pt[:],
                    scalar1=rpb2[:, m:m + 1], scalar2=None, op0=AL.add)
    if debug:
        with tc.tile_pool(name="dbg0", bufs=1) as dbg0, \
             tc.tile_pool(name="dbg0p", bufs=2, space="PSUM") as dbg0p:
            qpd = dbg0.tile([128, NQB, D], F32, name="qpd")
            for m in range(2):
                for a in range(NQB):
                    pt = dbg0p.tile([128, 128], F32, tag="ptr", name="ptr")
                    nc.tensor.transpose(out=pt[:],
                                        in_=qposT[m][:, a * 128:(a + 1) * 128],
                                        identity=ident[:])
                    nc.scalar.activation(out=qpd[:, a, m * 128:(m + 1) * 128],
                                         in_=pt[:], func=AF.Copy)
            nc.sync.dma_start(out=_qmaj(dbg["d_qpos"], D), in_=qpd[:])

    # ================= phase 1: value_in + gather tables =================
    vpw = consts.tile([128, NLAYERS, 2, HPC * HD], BF16)
    nc.sync.dma_start(out=vpw[:], in_=ins["vp_w"][:, :, :].rearrange(
        "l (c p) n -> p l c n", p=128))
    ztile = consts.tile([128, HD], BF16)
    nc.vector.memset(ztile[:], 0.0)
    with tc.tile_pool(name="vin", bufs=1) as vin_pool, \
         tc.tile_pool(name="vwork", bufs=3) as vwork, \
         tc.tile_pool(name="vpsum", bufs=4, space="PSUM") as vpsum:
        vin = [vin_pool.tile([128, LEN], BF16, tag=f"vin{c}", name=f"vin{c}") for c in range(2)]
        for c in range(2):
            nc.sync.dma_start(out=vin[c][:],
                              in_=ins["vin"][c * 128:(c + 1) * 128, :])
        for i in range(NLAYERS):
            for s0 in range(0, LEN, 1024):
                nb = min(1024, LEN - s0) // 128
                pt = vpsum.tile([128, 8, HPC * HD], F32, tag="vp", name="vp")
                for t in range(nb):
                    for c in range(2):
                        lhsT = _ap(vin[c], s0 + t, [_p0(vin[c]), [nb, 128]])
                        nc.tensor.matmul(out=pt[:, t], lhsT=lhsT,
                                         rhs=vpw[:, i, c, :],
                                         start=(c == 0), stop=(c == 1))
                st = vwork.tile([128, 8, HPC * HD], BF16, tag="st", name="st")
                nc.scalar.activation(out=st[:, :nb], in_=pt[:, :nb],
                                     func=AF.Identity)
                for h in range(HPC):
                    for cp in range(4):
                        off = (s0 + 1 - cp) * HD + cp * RPC * ELEM
                        eng = (nc.sync, nc.scalar)[(h * 4 + cp) % 2]
                        eng.dma_start(
                            out=bass.AP(tables[i][h].tensor, off,
                                        [[nb * HD, 128], [HD, nb], [1, HD]]),
                            in_=st[:, :nb, h * HD:(h + 1) * HD])
            for h in range(HPC):
                for cp in range(4):
                    if cp == 0:
                        nc.sync.dma_start(
                            out=bass.AP(tables[i][h].tensor, 0,
                                        [[HD, 1], [1, HD]]),
                            in_=ztile[:1, :])
                    lo = LEN + 1
                    n_pad = cp + 3
                    nc.sync.dma_start(
                        out=bass.AP(tables[i][h].tensor,
                                    cp * RPC * ELEM + (lo - cp) * HD,
                                    [[HD, n_pad], [1, HD]]),
                        in_=ztile[:n_pad, :])

    # ================= main-loop pools + weights =================
    lwork = ctx.enter_context(tc.tile_pool(name="lwork", bufs=1))
    big = ctx.enter_context(tc.tile_pool(name="big", bufs=4))
    psum = ctx.enter_context(tc.tile_pool(name="psum", bufs=2, space="PSUM"))
    soaw_w = consts.tile([128, NLAYERS, 2, 96], F32)
    nc.sync.dma_start(out=soaw_w[:], in_=ins["soaw_w"][:, :, :].rearrange(
        "l (c p) n -> p l c n", p=128))
    soaw_b = consts.tile([128, NLAYERS, 96], F32)
    nc.sync.dma_start(out=soaw_b[:], in_=bass.AP(
        ins["soaw_b"].ap().tensor, 0, [[0, 128], [96, NLAYERS], [1, 96]]))
    opw = consts.tile([68, NLAYERS, D], F32)
    nc.sync.dma_start(out=opw[:], in_=ins["opw_aug"][:, :, :].rearrange(
        "l p n -> p l n"))
    f1bT = consts.tile([128, NLAYERS, 8], F32)
    nc.sync.dma_start(out=f1bT[:], in_=bass.AP(
        ins["f1_b"].ap().tensor, 0, [[1, 128], [DFF, NLAYERS], [128, 8]]))
    fwins = (ins["f1_w"], ins["f2_w"])
    fbins = {nm: ins[nm] for nm in ("f2_b", "n1_s", "n1_b", "n2_s", "n2_b")}

    src = persist.tile([128, NQB, D], F32)
    nc.sync.dma_start(out=src[:], in_=ins["tgts"][:, :].rearrange(
        "(a p) d -> p a d", p=128))
    eps = consts.tile([128, 1], F32)
    nc.vector.memset(eps[:], 1e-5)

    for li in range(NLAYERS):
        _layer(nc, li, src, qposT, soaw_w, soaw_b, opw, fwins, f1bT, fbins,
               cw, locc, ident, eps, tables[li], dwork,
               groups, lwork, big, psum, dbg, debug and li == 0)

    nc.sync.dma_start(out=_qmaj(out, D), in_=src[:])
    if debug:
        nc.gpsimd.dma_start(out=dbg["d_val"][:, :],
                            in_=bass.AP(tables[0][0].tensor, 0,
                                        [[ELEM, 128], [1, ELEM]]))
    ctx.close()


def _layer(nc, li, src, qposT, soaw_w, soaw_b, opw, fwins, f1bT, fbins,
           cw, locc, ident, eps, tbls, dwork, groups,
           lwork, big, psum, dbg, dump):
    P4 = [128, NQB, 2, 16]
    # per-layer FFN weights + bias rows
    f1w = lwork.tile([128, 2, DFF], BF16, tag="f1w", name="f1w")
    nc.sync.dma_start(out=f1w[:], in_=fwins[0][li, :, :].rearrange(
        "(c p) n -> p c n", p=128))
    f2w = lwork.tile([128, 8, D], BF16, tag="f2w", name="f2w")
    nc.sync.dma_start(out=f2w[:], in_=fwins[1][li, :, :].rearrange(
        "(c p) n -> p c n", p=128))
    fb = {}
    for nm, dr in fbins.items():
        fb[nm] = lwork.tile([128, D], F32, tag=f"fb_{nm}", name=f"fb_{nm}")
        nc.sync.dma_start(out=fb[nm][:], in_=bass.AP(
            dr.ap().tensor, li * D, [[0, 128], [1, D]]))

    def T(tag, shape=None, dtp=F32):
        return lwork.tile(shape or P4, dtp, tag=tag, name=tag)

    def tt(o, i0, i1, op):
        nc.any.tensor_tensor(out=o, in0=i0, in1=i1, op=op)

    # ---- qT = srcT + qposT ----
    qT = [T(f"qT{m}", [128, NQ]) for m in range(2)]
    for a in range(NQB):
        for m in range(2):
            pt = psum.tile([128, 128], F32, tag="ptr", name="ptr")
            nc.tensor.transpose(out=pt[:], in_=src[:, a, m * 128:(m + 1) * 128],
                                identity=ident[:])
            nc.scalar.activation(out=qT[m][:, a * 128:(a + 1) * 128],
                                 in_=pt[:], func=AF.Copy)
    for m in range(2):
        tt(qT[m][:], qT[m][:], qposT[m][:], AL.add)

    # ---- so/aw ----
    soaw = T("soaw", [128, NQB, 96])
    for a in range(NQB):
        pt = psum.tile([128, 96], F32, tag="p256", name="p256")
        for m in range(2):
            nc.tensor.matmul(out=pt[:], lhsT=qT[m][:, a * 128:(a + 1) * 128],
                             rhs=soaw_w[:, li, m, :], start=(m == 0),
                             stop=(m == 1))
        nc.vector.tensor_tensor(out=soaw[:, a], in0=pt[:], in1=soaw_b[:, li],
                                op=AL.add)
    if dump:
        nc.sync.dma_start(out=_qmaj(dbg["d_so"], 64), in_=_ap(
            soaw, 0, [_p0(soaw), [96, NQB], [1, 64]]))

    # ---- softmax over 16 per (q, h) ----
    aw = T("aw")
    mx = T("mx", [128, NQB, 2])
    awl = _ap(soaw, 64, [_p0(soaw), [96, NQB], [16, 2], [1, 16]])
    nc.vector.tensor_reduce(out=mx[:], in_=awl, axis=AX.X, op=AL.max)
    tt(aw[:], awl, _ap(mx, 0, [_p0(mx), [2, NQB], [1, 2], [0, 16]]),
       AL.subtract)
    nc.scalar.activation(out=aw[:], in_=aw[:], func=AF.Exp)
    sm = T("sm", [128, NQB, 2])
    nc.vector.tensor_reduce(out=sm[:], in_=aw[:], axis=AX.X, op=AL.add)
    nc.vector.reciprocal(out=sm[:], in_=sm[:])
    tt(aw[:], aw[:], _ap(sm, 0, [_p0(sm), [2, NQB], [1, 2], [0, 16]]), AL.mult)
    if dump:
        nc.sync.dma_start(out=_qmaj(dbg["d_aw"], 32), in_=_ap(
            aw, 0, [_p0(aw), [32, NQB], [1, 32]]))

    # ---- pixel coords ----
    sox = _ap(soaw, 0, [_p0(soaw), [96, NQB], [32, 2], [2, 16]])
    soy = _ap(soaw, 1, [_p0(soaw), [96, NQB], [32, 2], [2, 16]])

    def lc(ci):
        return _ap(locc, ci * NQB * 32,
                   [_p0(locc), [32, NQB], [0, 2], [1, 16]])

    def cwv(r):
        return _ap(cw, r * 32, [_p0(cw), [0, NQB], [16, 2], [1, 16]])

    x, y = T("x"), T("y")
    tt(x[:], sox, lc(1), AL.mult)
    tt(x[:], x[:], lc(0), AL.add)
    tt(y[:], soy, lc(3), AL.mult)
    tt(y[:], y[:], lc(2), AL.add)
    if dump:
        nc.sync.dma_start(out=_qmaj(dbg["d_x"], 32), in_=_ap(
            x, 0, [_p0(x), [32, NQB], [1, 32]]))
        nc.sync.dma_start(out=_qmaj(dbg["d_y"], 32), in_=_ap(
            y, 0, [_p0(y), [32, NQB], [1, 32]]))

    ti = T("ti", P4, I32)
    tf = T("tf")
    fx, fy = T("fx"), T("fy")

    def floor_(dst, xin):
        nc.vector.tensor_copy(out=ti[:], in_=xin)
        nc.vector.tensor_copy(out=tf[:], in_=ti[:])
        nc.vector.tensor_tensor(out=dst[:], in0=tf[:], in1=xin, op=AL.is_gt)
        nc.vector.tensor_tensor(out=dst[:], in0=tf[:], in1=dst[:],
                                op=AL.subtract)

    floor_(fx, x[:])
    floor_(fy, y[:])
    lx, ly = T("lx"), T("ly")
    tt(lx[:], x[:], fx[:], AL.subtract)
    tt(ly[:], y[:], fy[:], AL.subtract)

    t1 = T("t1")
    ix0, ix1, iy0, iy1 = T("iy0"), T("iy1"), T("iy2"), T("iy3")
    for (dst, f, hi) in ((ix0, fx, 3), (iy0, fy, 4)):
        nc.any.tensor_scalar(out=dst[:], in0=f[:], scalar1=0.0, scalar2=None,
                             op0=AL.is_ge)
        tt(t1[:], f[:], cwv(hi), AL.is_le)
        tt(dst[:], dst[:], t1[:], AL.mult)
    for (dst, f, hi) in ((ix1, fx, 5), (iy1, fy, -1)):
        nc.any.tensor_scalar(out=dst[:], in0=f[:], scalar1=-1.0, scalar2=None,
                             op0=AL.is_ge)
        if hi >= 0:
            tt(t1[:], f[:], cwv(hi), AL.is_le)          # fx <= W-2
        else:
            nc.any.tensor_scalar(out=t1[:], in0=f[:], scalar1=1.0,
                                 scalar2=None, op0=AL.add)
            tt(t1[:], t1[:], cwv(4), AL.is_le)          # fy+1 <= H-1
        tt(dst[:], dst[:], t1[:], AL.mult)

    wy0, wy1, wx0, wx1 = T("wy0"), T("wy1"), T("ix0"), T("ix1")
    nc.any.tensor_scalar(out=wy0[:], in0=ly[:], scalar1=-1.0, scalar2=-1.0,
                         op0=AL.add, op1=AL.mult)       # (ly-1)*-1
    tt(wy0[:], wy0[:], iy0[:], AL.mult)
    tt(wy0[:], wy0[:], aw[:], AL.mult)
    tt(wy1[:], ly[:], iy1[:], AL.mult)
    tt(wy1[:], wy1[:], aw[:], AL.mult)
    nc.any.tensor_scalar(out=wx0[:], in0=lx[:], scalar1=-1.0, scalar2=-1.0,
                         op0=AL.add, op1=AL.mult)
    tt(wx0[:], wx0[:], ix0[:], AL.mult)
    tt(wx1[:], lx[:], ix1[:], AL.mult)

    wall = T("wall", [128, NQB, 2, 16, 2, 2], BF16)
    for r, wy in ((0, wy0), (1, wy1)):
        for c, wx in ((0, wx0), (1, wx1)):
            tt(t1[:], wy[:], wx[:], AL.mult)
            nc.vector.tensor_copy(out=_ap(
                wall, r * 2 + c,
                [_p0(wall), [128, NQB], [64, 2], [4, 16]]), in_=t1[:])
    if dump:
        nc.gpsimd.dma_start(out=_qmaj(dbg["d_w"], 128), in_=_ap(
            wall, 0, [_p0(wall), [128, NQB], [1, 128]]))

    samp = T("samp", [128, NQB, 68])
    nc.vector.tensor_reduce(out=_ap(samp, 64, [_p0(samp), [68, NQB], [1, 2]]),
                            in_=_ap(wall, 0, [_p0(wall), [128, NQB], [64, 2],
                                              [1, 64]]),
                            axis=AX.X, op=AL.add)
    nc.vector.memset(samp[:, :, 66:68], 1.0)

    # ---- flat index -> table rows ----
    s0, s1 = T("x"), T("y")  # reuse dead slots
    tt(s0[:], fy[:], cwv(0), AL.mult)
    tt(s0[:], s0[:], fx[:], AL.add)
    tt(s0[:], s0[:], cwv(2), AL.add)
    tt(s1[:], s0[:], cwv(0), AL.add)
    if dump:
        nc.sync.dma_start(out=_qmaj(dbg["d_s0"], 32), in_=_ap(
            s0, 0, [_p0(s0), [32, NQB], [1, 32]]))
    idxe = T("idxe", [128, NQB, 2, 16, 2])
    for r, s in ((0, s0), (1, s1)):
        nc.any.tensor_scalar(out=s[:], in0=s[:], scalar1=0.0, scalar2=SMAX,
                             op0=AL.max, op1=AL.min)
        nc.any.tensor_scalar(out=t1[:], in0=s[:], scalar1=0.25, scalar2=-0.375,
                             op0=AL.mult, op1=AL.add)
        nc.vector.tensor_copy(out=ti[:], in_=t1[:])
        nc.vector.tensor_copy(out=t1[:], in_=ti[:])
        # c = s - 4*floor4 in [0,3]; idx = c*RPC + floor4  (all exact in f32)
        nc.vector.scalar_tensor_tensor(out=tf[:], in0=t1[:], scalar=-4.0,
                                       in1=s[:], op0=AL.mult, op1=AL.add)
        nc.vector.scalar_tensor_tensor(
            out=_ap(idxe, r, [_p0(idxe), [64, NQB], [32, 2], [2, 16]]),
            in0=tf[:], scalar=float(RPC), in1=t1[:],
            op0=AL.mult, op1=AL.add)
    if dump:
        nc.sync.dma_start(out=_qmaj(dbg["d_idx"], 64), in_=_ap(
            idxe, 0, [_p0(idxe), [64, NQB], [1, 64]]))

    # ---- idx wrap bridge ----
    idxT = T("qT0", [128, NQ])
    for a in range(NQB):
        pt = psum.tile([128, 128], F32, tag="ptr", name="ptr")
        nc.tensor.transpose(out=pt[:64, :],
                            in_=_ap(idxe, a * 64, [_p0(idxe), [1, 64]]),
                            identity=ident[:])
        nc.scalar.activation(out=idxT[:64, a * 128:(a + 1) * 128],
                             in_=pt[:64, :], func=AF.Copy)
    st16 = T("st16", [16, NQB, 2, 32, 8], I16)
    for aq in range(NQB):
        for a8 in range(8):
            pt = psum.tile([128, 128], F32, tag="ptr", name="ptr")
            nc.tensor.transpose(
                out=pt[:16, :64],
                in_=idxT[:64, aq * 128 + a8 * 16:aq * 128 + a8 * 16 + 16],
                identity=ident[:64, :64])
            nc.vector.tensor_copy(out=_ap(
                st16, aq * 512 + a8, [_p0(st16), [256, 2], [8, 32]]),
                in_=pt[:16, :64])
    idx_dram = dwork.tile([16, NQB * HPC * KP * 8], I16, tag="idxd",
                          name="idxd")
    nc.sync.dma_start(out=idx_dram[:, :], in_=st16[:].rearrange(
        "p a h k e -> p (a h k e)"))

    # ---- gather + weighted sum ----
    for a in range(NQB):
        for h in range(HPC):
            idx16 = big.tile([128, 256], I16, tag="idx16", name="idx16")
            nc.sync.dma_start(out=idx16[:], in_=_ap(
                idx_dram, a * 512 + h * 256,
                [[0, 8], [4096, 16], [1, 256]]))
            g = big.tile([128, KP, ELEM], BF16, tag="g", name="g")
            if SKIP_GATHER:
                nc.vector.memset(g[:], 0.25)
            else:
                nc.gpsimd.dma_gather(
                    out_ap=g[:], in_ap=tbls[h][:, :], idxs_ap=idx16[:],
                    num_idxs=128 * KP, num_idxs_reg=128 * KP, elem_size=ELEM,
                    single_packet=False, queue_num=(a * HPC + h) % 4)
            m = big.tile([128, KP, 64], BF16, tag="m", name="m")
            nc.any.tensor_tensor(
                out=m[:],
                in0=_ap(g, 0, [_p0(g), [ELEM, KP], [32, 2], [1, 32]]),
                in1=_ap(wall, a * 128 + h * 64,
                        [_p0(wall), [2, KP], [1, 2], [0, 32]]),
                op=AL.mult)
            s = big.tile([128, KP, 32], BF16, tag="s", name="s")
            nc.any.tensor_tensor(out=s[:], in0=m[:, :, 0:32],
                                 in1=m[:, :, 32:64], op=AL.add)
            nc.vector.tensor_reduce(
                out=samp[:, a, h * 32:(h + 1) * 32],
                in_=_ap(s, 0, [_p0(s), [1, 32], [32, KP]]),
                axis=AX.X, op=AL.add)
    if dump:
        nc.sync.dma_start(out=_qmaj(dbg["d_samp"], 68), in_=samp[:])

    # ---- sampledT + op projection + AllReduce ----
    sampT = T("qT1", [68, NQ])
    for a in range(NQB):
        pt = psum.tile([128, 128], F32, tag="ptr", name="ptr")
        nc.tensor.transpose(out=pt[:68, :], in_=samp[:, a, :],
                            identity=ident[:])
        nc.scalar.activation(out=sampT[:, a * 128:(a + 1) * 128],
                             in_=pt[:68, :], func=AF.Copy)
    attn = T("attn", [128, NQB, D])
    for a in range(NQB):
        pt = psum.tile([128, D], F32, tag="p256", name="p256")
        nc.tensor.matmul(out=pt[:], lhsT=sampT[:, a * 128:(a + 1) * 128],
                         rhs=opw[:, li, :], start=True, stop=True)
        nc.scalar.activation(out=attn[:, a], in_=pt[:], func=AF.Copy)
    if not SKIP_CC:
        cc_in = dwork.tile([NQ, D], F32, tag="cc_in", name="cc_in")
        cc_out = dwork.tile([NQ, D], F32, tag="cc_out", name="cc_out")
        nc.sync.dma_start(out=_qmaj(cc_in, D), in_=attn[:])
        nc.gpsimd.collective_compute(
            "AllReduce", AL.add, replica_groups=groups,
            ins=[cc_in[:].opt()], outs=[cc_out[:].opt()])
        nc.sync.dma_start(out=attn[:], in_=_qmaj(cc_out, D))
    if dump:
        nc.sync.dma_start(out=_qmaj(dbg["d_attn"], D), in_=attn[:])

    _layernorm(nc, src, attn, fb["n1_s"], fb["n1_b"], eps, lwork, src)
    if dump:
        nc.sync.dma_start(out=_qmaj(dbg["d_src1"], D), in_=src[:])

    # ---- FFN ----
    s1T = [T(f"s1T{m}", [128, NQ], BF16) for m in range(2)]
    for a in range(NQB):
        for m in range(2):
            pt = psum.tile([128, 128], F32, tag="ptr", name="ptr")
            nc.tensor.transpose(out=pt[:], in_=src[:, a, m * 128:(m + 1) * 128],
                                identity=ident[:])
            nc.scalar.activation(out=s1T[m][:, a * 128:(a + 1) * 128],
                                 in_=pt[:], func=AF.Copy)
    hT = [T(f"hT{m8}", [128, NQ], BF16) for m8 in range(8)]
    for m8 in range(8):
        for qc in range(2):
            pt = psum.tile([128, 512], F32, tag="p512", name="p512")
            for m in range(2):
                nc.tensor.matmul(out=pt[:],
                                 lhsT=f1w[:, m, m8 * 128:(m8 + 1) * 128],
                                 rhs=s1T[m][:, qc * 512:(qc + 1) * 512],
                                 start=(m == 0), stop=(m == 1))
            nc.scalar.activation(out=hT[m8][:, qc * 512:(qc + 1) * 512],
                                 in_=pt[:], func=AF.Relu,
                                 bias=f1bT[:, li, m8:m8 + 1])
    ffn = T("attn", [128, NQB, D])
    for a in range(NQB):
        pt = psum.tile([128, D], F32, tag="p256", name="p256")
        for m8 in range(8):
            nc.tensor.matmul(out=pt[:], lhsT=hT[m8][:, a * 128:(a + 1) * 128],
                             rhs=f2w[:, m8, :], start=(m8 == 0),
                             stop=(m8 == 7))
        nc.vector.tensor_tensor(out=ffn[:, a], in0=pt[:], in1=fb["f2_b"][:],
                                op=AL.add)
    _layernorm(nc, src, ffn, fb["n2_s"], fb["n2_b"], eps, lwork, src)


def _layernorm(nc, src, delta, gamma_t, beta_t, eps, lwork, out_tile):
    # delta <- src + delta;  out_tile <- LN(delta)*gamma + beta
    nc.any.tensor_tensor(out=delta[:], in0=src[:], in1=delta[:], op=AL.add)
    stats = lwork.tile([128, NQB, 6], F32, tag="ln_st", name="ln_st")
    mv = lwork.tile([128, NQB, 2], F32, tag="ln_mv", name="ln_mv")
    for a in range(NQB):
        nc.vector.bn_stats(out=stats[:, a], in_=delta[:, a])
        nc.vector.bn_aggr(out=mv[:, a], in_=stats[:, a])
    rstd = lwork.tile([128, NQB], F32, tag="ln_rs", name="ln_rs")
    nc.scalar.activation(out=rstd[:], in_=_ap(mv, 1, [_p0(mv), [2, NQB]]),
                         func=AF.Sqrt, bias=eps[:, 0:1])
    nc.vector.reciprocal(out=rstd[:], in_=rstd[:])
    nmr = lwork.tile([128, NQB], F32, tag="ln_nm", name="ln_nm")
    nc.vector.tensor_tensor(out=nmr[:], in0=_ap(mv, 0, [_p0(mv), [2, NQB]]),
                            in1=rstd[:], op=AL.mult)
    nc.any.tensor_scalar(out=nmr[:], in0=nmr[:], scalar1=-1.0, scalar2=None,
                         op0=AL.mult)
    for a in range(NQB):
        nc.vector.tensor_scalar(out=delta[:, a], in0=delta[:, a],
                                scalar1=rstd[:, a:a + 1],
                                scalar2=nmr[:, a:a + 1],
                                op0=AL.mult, op1=AL.add)
    g = _ap(gamma_t, 0, [_p0(gamma_t), [0, NQB], [1, D]])
    b = _ap(beta_t, 0, [_p0(beta_t), [0, NQB], [1, D]])
    nc.any.tensor_tensor(out=delta[:], in0=delta[:], in1=g, op=AL.mult)
    nc.any.tensor_tensor(out=out_tile[:], in0=delta[:], in1=b, op=AL.add)


# ======================= host side =======================

def _expand32(v_ql):
    return np.ascontiguousarray(
        np.broadcast_to(v_ql[:, None, :, None], (NQ, HPC, NL, NP))
        .reshape(NQ, 32)).astype(np.float32)


def _host_inputs(inputs):
    import ml_dtypes
    tgts = np.asarray(inputs["tgts"], np.float32)
    refp = np.asarray(inputs["reference_points"], np.float32)
    masks = [np.asarray(inputs[f"mask{l}"]) for l in range(NL)]
    vrs = []
    for m in masks:
        H, W = m.shape[1], m.shape[2]
        vh = (~m[:, :, 0]).sum(1).astype(np.float32) / H
        vw = (~m[:, 0, :]).sum(1).astype(np.float32) / W
        vrs.append(np.stack([vw, vh], -1))
    vr = np.stack(vrs, 1)

    i = np.arange(128, dtype=np.float64)
    dim_t = 10000.0 ** (2 * np.floor(i / 2) / 128)
    ds = np.stack([2 * math.pi / dim_t,
                   np.where(np.arange(128) % 2 == 0, 0.0, math.pi / 2)],
                  -1).astype(np.float32)

    cw = np.zeros((6, 32), np.float32)
    for h2 in range(HPC):
        for l in range(NL):
            sl = slice(h2 * 16 + l * NP, h2 * 16 + (l + 1) * NP)
            cw[0, sl] = LVL_W[l]
            cw[1, sl] = LVL_H[l]
            cw[2, sl] = LVL_BASE[l]
            cw[3, sl] = LVL_W[l] - 1
            cw[4, sl] = LVL_H[l] - 1
            cw[5, sl] = LVL_W[l] - 2

    so_w = np.asarray(inputs["so_w"], np.float32)
    so_b = np.asarray(inputs["so_b"], np.float32)
    aw_w = np.asarray(inputs["aw_w"], np.float32)
    aw_b = np.asarray(inputs["aw_b"], np.float32)
    vp_w = np.asarray(inputs["vp_w"], np.float32)
    vp_b = np.asarray(inputs["vp_b"], np.float32)
    op_w = np.asarray(inputs["op_w"], np.float32)
    op_b = np.asarray(inputs["op_b"], np.float32)
    Wv = np.array(LVL_W, np.float32)[None]
    Hv = np.array(LVL_H, np.float32)[None]

    lemb = np.asarray(inputs["level_embed"], np.float32)
    vin_b = []
    for b in range(BS):
        parts = []
        for l in range(NL):
            parts.append(
                np.asarray(inputs[f"mem{l}"], np.float32)[b].reshape(D, -1)
                + np.asarray(inputs[f"pm{l}"], np.float32)[b].reshape(D, -1)
                + lemb[l][:, None])
        vin_b.append(np.ascontiguousarray(
            np.concatenate(parts, 1)).astype(ml_dtypes.bfloat16))

    in_maps = []
    for core in range(NCORES):
        b, hg = core // 4, core % 4
        h0 = 2 * hg
        m = {}
        m["tgts"] = tgts[b]
        m["refc"] = np.ascontiguousarray(refp[b].T[[1, 0, 2, 3]])
        m["vin"] = vin_b[b]
        ri = refp[b][:, None, :] * np.concatenate([vr[b], vr[b]], -1)[None]
        m["ax"] = _expand32(ri[:, :, 0] * Wv - 0.5)
        m["bx"] = _expand32(ri[:, :, 2] * Wv * (0.5 / NP))
        m["ay"] = _expand32(ri[:, :, 1] * Hv - 0.5)
        m["by"] = _expand32(ri[:, :, 3] * Hv * (0.5 / NP))
        m["cw"] = cw
        m["ds"] = ds
        m["rp_w1"] = np.asarray(inputs["rp_w1"], np.float32)
        m["rp_b1"] = np.asarray(inputs["rp_b1"], np.float32)
        m["rp_w2"] = np.asarray(inputs["rp_w2"], np.float32)
        m["rp_b2"] = np.asarray(inputs["rp_b2"], np.float32)
        so_r = so_w.reshape(NLAYERS, D, NH, NL, NP, 2)
        aw_r = aw_w.reshape(NLAYERS, D, NH, NL, NP)
        sob_r = so_b.reshape(NLAYERS, NH, NL, NP, 2)
        awb_r = aw_b.reshape(NLAYERS, NH, NL, NP)
        m["soaw_w"] = np.ascontiguousarray(np.concatenate([
            so_r[:, :, h0:h0 + 2].reshape(NLAYERS, D, 64),
            aw_r[:, :, h0:h0 + 2].reshape(NLAYERS, D, 32)], -1))
        m["soaw_b"] = np.ascontiguousarray(np.concatenate([
            sob_r[:, h0:h0 + 2].reshape(NLAYERS, 64),
            awb_r[:, h0:h0 + 2].reshape(NLAYERS, 32)], -1))
        m["vp_w"] = np.ascontiguousarray(vp_w[:, :, h0 * HD:(h0 + 2) * HD]).astype(ml_dtypes.bfloat16)
        opa = np.zeros((NLAYERS, 68, D), np.float32)
        for lii in range(NLAYERS):
            opa[lii, 0:64] = op_w[lii, h0 * HD:(h0 + 2) * HD]
            for hh in range(HPC):
                sl = slice((h0 + hh) * HD, (h0 + hh + 1) * HD)
                opa[lii, 64 + hh] = vp_b[lii, sl] @ op_w[lii, sl]
            opa[lii, 66] = op_b[lii] / 4.0
        m["opw_aug"] = opa
        m["f1_w"] = np.asarray(inputs["f1_w"], np.float32).astype(
            ml_dtypes.bfloat16)
        m["f2_w"] = np.asarray(inputs["f2_w"], np.float32).astype(
            ml_dtypes.bfloat16)
        m["f1_b"] = np.asarray(inputs["f1_b"], np.float32)
        m["f2_b"] = np.asarray(inputs["f2_b"], np.float32)
        for nm in ("n1_s", "n1_b", "n2_s", "n2_b"):
            m[nm] = np.asarray(inputs[nm], np.float32)
        in_maps.append(m)
    return in_maps


def kernel(**inputs):
    if "nc" not in _CACHE:
        _CACHE["nc"] = build_nc(debug=False)
    nc = _CACHE["nc"]
    in_maps = _host_inputs(inputs)
    res = run_bass_kernel_spmd(nc, in_maps, list(range(NCORES)))
    return np.stack([res.results[0]["out"],
                     res.results[4]["out"]]).astype(np.float32)

